# revision 6
# baseline (speedup 1.0000x reference)
"""Linformer self-attention Trainium2 kernel (fp16 PE pipeline).

Problem (hardcoded): B=4, N=4096, DIM=1024, H=16, K=256, HD=64, fp32 I/O.
  qkv = x @ Wqkv.T; q,k,v split into 16 heads of 64
  k_proj = E @ k, v_proj = F @ v  (per head, contract over tokens)
  out = softmax(q @ k_proj.T / 8) @ v_proj
  y = out @ Wout.T + bout

Sharding: 8 cores = (batch b = c//2) x (head-group hg = c%2, 8 heads each).
Each core computes a (4096, 1024) fp16 partial of y for its batch; host
sums hg=0 + hg=1 partials in fp32 and adds bout. No collectives.

All matmuls run in fp16. E is pre-scaled by 1/8 on the host so scores
come out of the PE pre-scaled and the negated max feeds exp's bias
directly. Per core:
  Phase A: as before (k,v per 128-token chunk contracted with streamed
    E/F into k_projT/v_projT psum accumulators). Startup split across
    the SP and ACT DGE queues so the first matmul starts earlier.
  Phase C (software-pipelined over 32 head-pair slots), engine-balanced:
    scores (PE) -> batched DVE max -> ACT exp (bias = negated max, no
    accumulator) -> DVE den sums (one 3D reduce per exp tile into a
    shared [128,8] tile) -> one DVE reciprocal -> gpsimd normalize.
    The fp16 attn transpose is split: token-half 0 goes through the DMA
    XBAR (dma_start_transpose on SP), token-half 1 through PE transpose
    matmuls + one DVE psum copy; both land in one at2 [128 K, 2048]
    tile per pair. AV issues DEPTH slots later. The output projection
    runs ec-blocked (4 matmuls + copy per 512-wide half) so the psum
    bank for ec0 recycles while ec1's matmuls run; y copies split
    ACT(ec0)/DVE(ec1); q copies on ACT.
"""

import numpy as np

B, N, DIM, H, K = 4, 4096, 1024, 16, 256
HD = DIM // H
SCALE = 1.0 / 8.0
HG = H // 2          # 8 heads per core
HGD = HG * HD        # 512 head dims per core
NCORES = 8
TG = 512             # token group
NTC = N // 128       # 32 token chunks
NGRP = N // TG       # 8 token groups

_cache = {}


def _build(level=4):
    import concourse.mybir as mybir
    import concourse.tile as tile
    from concourse import bacc
    from concourse.masks import make_identity

    f32 = mybir.dt.float32
    f16 = mybir.dt.float16
    AX = mybir.AxisListType.X
    MAX = mybir.AluOpType.max
    ADD = mybir.AluOpType.add
    EXP = mybir.ActivationFunctionType.Exp

    nc = bacc.Bacc("TRN2", target_bir_lowering=False, debug=False,
                   enable_asserts=False)

    xT = nc.dram_tensor("xT", (DIM, N), f16, kind="ExternalInput").ap()
    wqT = nc.dram_tensor("wqT", (DIM, HGD), f16, kind="ExternalInput").ap()
    wkvT = nc.dram_tensor("wkvT", (DIM, 2 * HGD), f16, kind="ExternalInput").ap()
    eT = nc.dram_tensor("eT", (N, HG * K), f16, kind="ExternalInput").ap()
    fT = nc.dram_tensor("fT", (N, HG * K), f16, kind="ExternalInput").ap()
    woutT = nc.dram_tensor("woutT", (HGD, DIM), f16, kind="ExternalInput").ap()
    y = nc.dram_tensor("y", (N, DIM), f16, kind="ExternalOutput").ap()

    def hloc(h):
        # head h (0..7) -> (tile idx, partition base, free base) in the
        # packed [128, 512] projT psum/sbuf tiles
        sub = h % 4
        return h // 4, (sub % 2) * 64, (sub // 2) * 256

    st = {}

    def issue_q_quarter(g, qc):
        """One qdim-chunk of q for a future group -- PE filler work
        interleaved with other phases."""
        x_s, wq_s, q_s, psq = st["x_s"], st["wq_s"], st["q_s"], st["psq"]
        pq = psq.tile([128, 512], f32, name="pq")
        for dc in range(8):
            nc.tensor.matmul(
                pq[:],
                wq_s[:, dc * HGD + qc * 128: dc * HGD + (qc + 1) * 128],
                x_s[:, dc * N + g * TG: dc * N + (g + 1) * TG],
                start=(dc == 0), stop=(dc == 7))
        nc.scalar.copy(
            q_s[:, qc * N + g * TG: qc * N + (g + 1) * TG], pq[:])
    st["issue_q_quarter"] = issue_q_quarter

    def phase_kv(tc):
        efp, kvp = st["efp"], st["kvp"]
        pskv, pspr = st["pskv"], st["pspr"]
        x_s, wkv_s = st["x_s"], st["wkv_s"]
        projp = st["projp"]

        kprojT_ps = [pspr.tile([128, 512], f32, name=f"kprojTps{i}")
                     for i in range(2)]
        vprojT_ps = [pspr.tile([128, 512], f32, name=f"vprojTps{i}")
                     for i in range(2)]

        wq_s, wout_s = st["wq_s"], st["wout_s"]
        for tc_i in range(NTC):
            if tc_i == 12:
                for dc in range(8):
                    nc.sync.dma_start(wq_s[:, dc * HGD:(dc + 1) * HGD],
                                      wqT[dc * 128:(dc + 1) * 128, :])
            if tc_i == 16:
                for dc in range(4):
                    nc.sync.dma_start(wout_s[:, dc * DIM:(dc + 1) * DIM],
                                      woutT[dc * 128:(dc + 1) * 128, :])

            if tc_i == 0:
                ef0 = st["ef0"]
                eg = ef0[:, 0:HG * K]
                fg = ef0[:, HG * K:2 * HG * K]
            else:
                eg_t = efp.tile([128, HG * K], f16, name="eg")
                fg_t = efp.tile([128, HG * K], f16, name="fg")
                nc.sync.dma_start(eg_t[:], eT[tc_i * 128:(tc_i + 1) * 128, :])
                nc.sync.dma_start(fg_t[:], fT[tc_i * 128:(tc_i + 1) * 128, :])
                eg, fg = eg_t[:], fg_t[:]
            # x prefetch AFTER the E/F issue so it never queues ahead of
            # the stream the proj matmuls wait on
            g2 = tc_i // 4 + 2
            if g2 < NGRP:
                for dc in (2 * (tc_i % 4), 2 * (tc_i % 4) + 1):
                    nc.sync.dma_start(
                        x_s[:, dc * N + g2 * TG: dc * N + (g2 + 1) * TG],
                        xT[dc * 128:(dc + 1) * 128, g2 * TG:(g2 + 1) * TG])
            pk = pskv.tile([128, 512], f32, name="pk")
            pv = pskv.tile([128, 512], f32, name="pv")
            for dc in range(8):
                xc = x_s[:, dc * N + tc_i * 128: dc * N + (tc_i + 1) * 128]
                nc.tensor.matmul(pk[:], xc,
                                 wkv_s[:, dc * 1024: dc * 1024 + 512],
                                 start=(dc == 0), stop=(dc == 7))
            kvt = kvp.tile([128, 1024], f16)
            nc.scalar.copy(kvt[:, 0:512], pk[:])
            for dc in range(8):
                xc = x_s[:, dc * N + tc_i * 128: dc * N + (tc_i + 1) * 128]
                nc.tensor.matmul(pv[:], xc,
                                 wkv_s[:, dc * 1024 + 512: dc * 1024 + 1024],
                                 start=(dc == 0), stop=(dc == 7))
            nc.scalar.copy(kvt[:, 512:1024], pv[:])
            # accumulate k_projT / v_projT over token chunks.
            # psum zero regions are 2KB per partition row: the two heads
            # sharing (tile, partition half) share one accumulation
            # group -> start on fb==0 head, stop on fb==256 head.
            for h in range(HG):
                i, pb, fb = hloc(h)
                nc.tensor.matmul(
                    kprojT_ps[i][pb:pb + 64, fb:fb + 256],
                    kvt[:, h * 64:(h + 1) * 64],
                    eg[:, h * K:(h + 1) * K],
                    start=(tc_i == 0 and fb == 0),
                    stop=(tc_i == NTC - 1 and fb == 256),
                    skip_group_check=True)
            for h in range(HG):
                i, pb, fb = hloc(h)
                nc.tensor.matmul(
                    vprojT_ps[i][pb:pb + 64, fb:fb + 256],
                    kvt[:, 512 + h * 64: 512 + (h + 1) * 64],
                    fg[:, h * K:(h + 1) * K],
                    start=(tc_i == 0 and fb == 0),
                    stop=(tc_i == NTC - 1 and fb == 256),
                    skip_group_check=True)

        kprojT_sb = [projp.tile([128, 512], f16, name=f"kprojT{i}")
                     for i in range(2)]
        vprojT_sb = [projp.tile([128, 512], f16, name=f"vprojT{i}")
                     for i in range(2)]
        for i in range(2):
            nc.scalar.copy(kprojT_sb[i][:], kprojT_ps[i][:])
            nc.scalar.copy(vprojT_sb[i][:], vprojT_ps[i][:])
        st["kprojT_sb"] = kprojT_sb
        st["vprojT_sb"] = vprojT_sb

    def phase_vp(tc):
        ident, projp = st["ident"], st["projp"]
        vprojT_sb, psvp = st["vprojT_sb"], st["psvp"]
        # vproj_sb[kc]: [128 K-chunk, 8 heads * 64 hd] fp16
        vproj_sb = [projp.tile([128, 512], f16, name=f"vproj{i}")
                    for i in range(2)]
        pvp = [psvp.tile([128, 512], f16, name=f"pvp{kc}") for kc in range(2)]
        for i in range(2):
            for c in range(4):
                h0 = 4 * i + (c // 2) * 2
                nc.tensor.transpose(
                    pvp[c % 2][:, h0 * 64: h0 * 64 + 128],
                    vprojT_sb[i][:, c * 128:(c + 1) * 128],
                    ident[:])
        for kc in range(2):
            nc.scalar.copy(vproj_sb[kc][:], pvp[kc][:])
        st["vproj_sb"] = vproj_sb

    def phase_attn(tc):
        ident = st["ident"]
        kprojT_sb, vproj_sb = st["kprojT_sb"], st["vproj_sb"]
        q_s, wout_s = st["q_s"], st["wout_s"]
        pep, at2p, hgp, ysbp, vecp = (st["pep"], st["at2p"], st["hgp"],
                                      st["ysbp"], st["vecp"])
        pss, psat, psho, psy = st["pss"], st["psat"], st["psho"], st["psy"]

        def issue_scores_softmax(g, hp):
            """Scores + softmax + transpose for one head pair; the
            normalized fp16 attn lands transposed in at2
            [128 K-part, (hs,kc) blocks of 512, token]."""
            at2 = at2p.tile([128, 2048], f16, name="at2")
            at2_3d = at2[:].rearrange("p (j c) -> p j c", c=512)
            den8 = vecp.tile([128, 8], f32, name="den8")
            r8 = vecp.tile([128, 8], f32, name="r8")
            pes = []
            for half in range(2):
                # one psum bank per head (a bank must only ever be
                # written from one PE row position -- mixing faults HW)
                pe2 = [pep.tile([128, 512], f16, name=f"pe{j}",
                                tag="pe") for j in range(2)]
                for hs in range(2):
                    h = hp * 2 + hs
                    i, pb, fb = hloc(h)
                    qc = h // 2
                    ps = pss.tile([128, 512], f32, name="ps")
                    for t2 in range(2):
                        t = half * 2 + t2
                        nc.tensor.matmul(
                            ps[:, t2 * 256:(t2 + 1) * 256],
                            q_s[pb:pb + 64,
                                qc * N + g * TG + t * 128:
                                qc * N + g * TG + (t + 1) * 128],
                            kprojT_sb[i][pb:pb + 64, fb:fb + 256],
                            start=True, stop=True)
                    nm = vecp.tile([128, 2], f32, name="nm")
                    nc.vector.tensor_reduce(
                        nm[:], ps[:].rearrange("p (c k) -> p c k", k=256),
                        axis=AX, op=MAX, negate=True)
                    for t2 in range(2):
                        nc.scalar.activation(
                            pe2[t2][:, hs * 256:(hs + 1) * 256],
                            ps[:, t2 * 256:(t2 + 1) * 256],
                            EXP, bias=nm[:, t2:t2 + 1])
                pes.extend(pe2)
            # den per (tile t, head hs): one 3D reduce per exp tile
            for t in range(4):
                nc.vector.tensor_reduce(
                    den8[:, t * 2:(t + 1) * 2],
                    pes[t][:].rearrange("p (c k) -> p c k", k=256),
                    axis=AX, op=ADD)
            nc.vector.reciprocal(r8[:], den8[:])
            # normalize on gpsimd (SBUF fp16, per-partition fp32 scalar)
            for t in range(4):
                for hs in range(2):
                    nc.gpsimd.tensor_scalar_mul(
                        pes[t][:, hs * 256:(hs + 1) * 256],
                        pes[t][:, hs * 256:(hs + 1) * 256],
                        r8[:, t * 2 + hs: t * 2 + hs + 1])
            # transpose: token-half 0 via DMA XBAR (SP), half 1 via PE
            for t2 in range(2):
                nc.sync.dma_start_transpose(
                    at2_3d[:, :, t2 * 128:(t2 + 1) * 128], pes[t2][:])
            pat = psat.tile([128, 1024], f16, name="pat")
            for t2 in range(2):
                for hs in range(2):
                    for kc in range(2):
                        nc.tensor.matmul(
                            pat[:, (hs * 2 + kc) * 256 + t2 * 128:
                                (hs * 2 + kc) * 256 + (t2 + 1) * 128],
                            pes[2 + t2][:, hs * 256 + kc * 128:
                                        hs * 256 + (kc + 1) * 128],
                            ident[:],
                            is_transpose=True, start=True, stop=True)
            nc.vector.tensor_copy(
                at2_3d[:, :, 256:512],
                pat[:].rearrange("p (j c) -> p j c", c=256))
            return at2

        def issue_av(g, hp, at2, hgt):
            """AV matmuls for a pair whose transposed attn is in at2."""
            pho = psho.tile([128, 512], f32, name="pho")
            for hs in range(2):
                h = hp * 2 + hs
                for kc in range(2):
                    nc.tensor.matmul(
                        pho[hs * 64:(hs + 1) * 64, :],
                        vproj_sb[kc][:, h * 64:(h + 1) * 64],
                        at2[:, (hs * 2 + kc) * 512:(hs * 2 + kc + 1) * 512],
                        start=(kc == 0), stop=(kc == 1),
                        skip_group_check=True)
            nc.vector.tensor_copy(hgt[hp][:], pho[:])

        issue_q_quarter = st["issue_q_quarter"]

        def issue_y(g, hgt):
            """Fused output projection for a finished token group,
            ec-blocked so the ec0 psum bank recycles during ec1's
            matmuls; copies split ACT(ec0)/DVE(ec1)."""
            for t in range(4):
                ysb = ysbp.tile([128, 1024], f16, name="ysb")
                for ec in range(2):
                    py = psy.tile([128, 512], f32, name="py")
                    for hp in range(4):
                        nc.tensor.matmul(
                            py[:],
                            hgt[hp][:, t * 128:(t + 1) * 128],
                            wout_s[:, hp * DIM + ec * 512:
                                   hp * DIM + (ec + 1) * 512],
                            start=(hp == 0), stop=(hp == 3))
                    if ec == 0:
                        nc.scalar.copy(ysb[:, 0:512], py[:])
                    else:
                        nc.vector.tensor_copy(ysb[:, 512:1024], py[:])
                nc.sync.dma_start(
                    y[(g * 4 + t) * 128:(g * 4 + t + 1) * 128, :], ysb[:])

        # software-pipelined: scores/softmax/transpose of pair i issue
        # before the AV of pair i-DEPTH, so the PE never waits on the
        # softmax chain; y projection of a group issues right after its
        # last AV.
        from collections import deque
        hgts = {}
        queue = deque()
        y_ready = None
        DEPTH = 4
        for qc in range(4):
            issue_q_quarter(0, qc)
        for qc in range(4):
            issue_q_quarter(1, qc)
        pairs = [(g, hp) for g in range(NGRP) for hp in range(4)]
        for g, hp in pairs + [(None, None)] * (DEPTH + 1):
            if g is not None:
                if hp == 0:
                    hgts[g] = [hgp.tile([128, TG], f16, name=f"hgt{i}")
                               for i in range(4)]
                queue.append((g, hp, issue_scores_softmax(g, hp)))
                if g + 2 < NGRP:
                    issue_q_quarter(g + 2, hp)
            if len(queue) > DEPTH or (g is None and queue):
                pg, php, pat2 = queue.popleft()
                issue_av(pg, php, pat2, hgts[pg])
                if php == 3:
                    y_ready = pg
                    continue
            if y_ready is not None:
                issue_y(y_ready, hgts.pop(y_ready))
                y_ready = None


    with tile.TileContext(nc) as tc:
        with (
            tc.tile_pool(name="const", bufs=1) as constp,
            tc.tile_pool(name="persist", bufs=1) as persistp,
            tc.tile_pool(name="proj_sb", bufs=1) as projp,
        ):
            ident = constp.tile([128, 128], f16)
            make_identity(nc, ident[:])
            st["ident"] = ident
            st["projp"] = projp

            x_s = persistp.tile([128, 8 * N], f16, name="x_s")
            wq_s = persistp.tile([128, 8 * HGD], f16, name="wq_s")
            wkv_s = persistp.tile([128, 8 * 2 * HGD], f16, name="wkv_s")
            wout_s = persistp.tile([128, 4 * DIM], f16, name="wout_s")
            q_s = persistp.tile([128, 4 * N], f16, name="q_s")
            st.update(x_s=x_s, wq_s=wq_s, wkv_s=wkv_s, wout_s=wout_s, q_s=q_s)

            def load_x_group(g):
                for dc in range(8):
                    nc.sync.dma_start(
                        x_s[:, dc * N + g * TG: dc * N + (g + 1) * TG],
                        xT[dc * 128:(dc + 1) * 128, g * TG:(g + 1) * TG])

            # startup: wkv-k split across the SP and ACT DGE queues and
            # x chunk 0 on ACT, so the first chunk's matmuls wait on two
            # parallel streams instead of one
            for dc in range(4):
                nc.sync.dma_start(wkv_s[:, dc * 2 * HGD: dc * 2 * HGD + HGD],
                                  wkvT[dc * 128:(dc + 1) * 128, 0:HGD])
            for dc in range(4, 8):
                nc.scalar.dma_start(wkv_s[:, dc * 2 * HGD: dc * 2 * HGD + HGD],
                                    wkvT[dc * 128:(dc + 1) * 128, 0:HGD])
            for dc in range(8):
                nc.scalar.dma_start(x_s[:, dc * N: dc * N + 128],
                                    xT[dc * 128:(dc + 1) * 128, 0:128])
            for dc in range(8):
                nc.sync.dma_start(x_s[:, dc * N + 128: dc * N + TG],
                                  xT[dc * 128:(dc + 1) * 128, 128:TG])
            st["ef0"] = ef0 = projp.tile([128, 2 * HG * K], f16, name="ef0")
            nc.scalar.dma_start(ef0[:, 0:HG * K], eT[0:128, :])
            for dc in range(8):
                nc.sync.dma_start(
                    wkv_s[:, dc * 2 * HGD + HGD:(dc + 1) * 2 * HGD],
                    wkvT[dc * 128:(dc + 1) * 128, HGD:2 * HGD])
            nc.scalar.dma_start(ef0[:, HG * K:2 * HG * K], fT[0:128, :])
            load_x_group(1)
            st["load_x_group"] = load_x_group

            psq_ctx = tc.tile_pool(name="ps_q", bufs=1, space="PSUM")
            st["psq"] = psq_ctx.__enter__()
            with (
                tc.tile_pool(name="ef", bufs=3) as efp,
                tc.tile_pool(name="kv", bufs=3) as kvp,
                tc.tile_pool(name="ps_kv", bufs=1, space="PSUM") as pskv,
                tc.tile_pool(name="ps_proj", bufs=1, space="PSUM") as pspr,
            ):
                st.update(efp=efp, kvp=kvp, pskv=pskv, pspr=pspr)
                phase_kv(tc)

            if level >= 2:
                with tc.tile_pool(name="ps_vp", bufs=1, space="PSUM") as psvp:
                    st["psvp"] = psvp
                    phase_vp(tc)

            if level >= 4:
              with (
                tc.tile_pool(name="pe", bufs=16) as pep,
                tc.tile_pool(name="at2", bufs=5) as at2p,
                tc.tile_pool(name="hgt", bufs=3) as hgp,
                tc.tile_pool(name="ysb", bufs=3) as ysbp,
                tc.tile_pool(name="vec", bufs=16) as vecp,
                tc.tile_pool(name="ps_s", bufs=2, space="PSUM") as pss,
                tc.tile_pool(name="ps_at", bufs=2, space="PSUM") as psat,
                tc.tile_pool(name="ps_ho", bufs=1, space="PSUM") as psho,
                tc.tile_pool(name="ps_y", bufs=2, space="PSUM") as psy,
            ):
                st.update(pep=pep, at2p=at2p, hgp=hgp, ysbp=ysbp, vecp=vecp,
                          pss=pss, psat=psat, psho=psho, psy=psy)
                phase_attn(tc)

            psq_ctx.__exit__(None, None, None)

    if level < 4:
        with tile.TileContext(nc) as tc2:
            with tc2.tile_pool(name="dummy", bufs=1) as dp:
                zt = dp.tile([128, DIM], f16)
                nc.gpsimd.memset(zt[:], 0.0)
                for gb in range(NTC):
                    nc.sync.dma_start(y[gb * 128:(gb + 1) * 128, :], zt[:])

    nc.compile()
    return nc


def _prep_inputs(x, Wqkv, E, F, Wout):
    """Build the 8 per-core input dicts (host-side slicing/transposes)."""
    f16 = np.float16
    ins = []
    per_hg = {}
    for hg in range(2):
        r = hg * HGD
        wqT = np.ascontiguousarray(Wqkv[r:r + HGD, :].T, dtype=f16)
        wk = Wqkv[DIM + r: DIM + r + HGD, :]
        wv = Wqkv[2 * DIM + r: 2 * DIM + r + HGD, :]
        wkvT = np.concatenate([wk.T, wv.T], axis=1).astype(f16)
        # E pre-scaled by SCALE so scores come out of the PE pre-scaled
        eT = np.ascontiguousarray(
            E[hg * HG:(hg + 1) * HG].transpose(2, 0, 1).reshape(N, HG * K)
            * SCALE, dtype=f16)
        fT = np.ascontiguousarray(
            F[hg * HG:(hg + 1) * HG].transpose(2, 0, 1).reshape(N, HG * K),
            dtype=f16)
        woutT = np.ascontiguousarray(Wout[:, r:r + HGD].T, dtype=f16)
        per_hg[hg] = (wqT, wkvT, eT, fT, woutT)
    xTs = [np.ascontiguousarray(x[b].T, dtype=f16) for b in range(B)]
    for c in range(NCORES):
        b, hg = c // 2, c % 2
        wqT, wkvT, eT, fT, woutT = per_hg[hg]
        ins.append({"xT": xTs[b], "wqT": wqT, "wkvT": wkvT,
                    "eT": eT, "fT": fT, "woutT": woutT})
    return ins


def kernel(x, Wqkv, E, F, Wout, bout):
    from concourse.bass_utils import run_bass_kernel_spmd

    x = np.asarray(x, dtype=np.float32)
    Wqkv = np.asarray(Wqkv, dtype=np.float32)
    E = np.asarray(E, dtype=np.float32)
    F = np.asarray(F, dtype=np.float32)
    Wout = np.asarray(Wout, dtype=np.float32)
    bout = np.asarray(bout, dtype=np.float32)

    if "nc" not in _cache:
        _cache["nc"] = _build()
    nc = _cache["nc"]

    in_maps = _prep_inputs(x, Wqkv, E, F, Wout)
    res = run_bass_kernel_spmd(nc, in_maps, core_ids=list(range(NCORES)))
    out = np.empty((B, N, DIM), dtype=np.float32)
    for b in range(B):
        out[b] = (res.results[2 * b]["y"].astype(np.float32)
                  + res.results[2 * b + 1]["y"].astype(np.float32) + bout)
    return out


# revision 7
# speedup vs baseline: 2.7762x; 2.7762x over previous
"""Linformer self-attention Trainium2 kernel (fp16 PE pipeline).

Problem (hardcoded): B=4, N=4096, DIM=1024, H=16, K=256, HD=64, fp32 I/O.
  qkv = x @ Wqkv.T; q,k,v split into 16 heads of 64
  k_proj = E @ k, v_proj = F @ v  (per head, contract over tokens)
  out = softmax(q @ k_proj.T / 8) @ v_proj
  y = out @ Wout.T + bout

Sharding: 8 cores = (batch b = c//2) x (head-group hg = c%2, 8 heads each).
Each core computes a (4096, 1024) fp16 partial of y for its batch; host
sums hg=0 + hg=1 partials in fp32 and adds bout. No collectives.

All matmuls run in fp16. E is pre-scaled by 1/8 on the host so scores
come out of the PE pre-scaled and the negated max feeds exp's bias
directly. Per core:
  Phase A: as before (k,v per 128-token chunk contracted with streamed
    E/F into k_projT/v_projT psum accumulators). Startup split across
    the SP and ACT DGE queues so the first matmul starts earlier.
  Phase C (software-pipelined over 32 head-pair slots), engine-balanced:
    scores (PE) -> batched DVE max -> ACT exp (bias = negated max, no
    accumulator) -> DVE den sums (one 3D reduce per exp tile into a
    shared [128,8] tile) -> one DVE reciprocal -> gpsimd normalize.
    The fp16 attn transpose is split: token-half 0 goes through the DMA
    XBAR (dma_start_transpose on SP), token-half 1 through PE transpose
    matmuls + one DVE psum copy; both land in one at2 [128 K, 2048]
    tile per pair. AV issues DEPTH slots later. The output projection
    runs ec-blocked (4 matmuls + copy per 512-wide half) so the psum
    bank for ec0 recycles while ec1's matmuls run; y copies split
    ACT(ec0)/DVE(ec1); q copies on ACT.
"""

import numpy as np

B, N, DIM, H, K = 4, 4096, 1024, 16, 256
HD = DIM // H
SCALE = 1.0 / 8.0
HG = H // 2          # 8 heads per core
HGD = HG * HD        # 512 head dims per core
NCORES = 8
TG = 512             # token group
NTC = N // 128       # 32 token chunks
NGRP = N // TG       # 8 token groups

_cache = {}


def _build(level=4):
    import concourse.mybir as mybir
    import concourse.tile as tile
    from concourse import bacc
    from concourse.masks import make_identity

    f32 = mybir.dt.float32
    f16 = mybir.dt.float16
    AX = mybir.AxisListType.X
    MAX = mybir.AluOpType.max
    ADD = mybir.AluOpType.add
    EXP = mybir.ActivationFunctionType.Exp

    nc = bacc.Bacc("TRN2", target_bir_lowering=False, debug=False,
                   enable_asserts=False)

    xT = nc.dram_tensor("xT", (DIM, N), f16, kind="ExternalInput").ap()
    wqT = nc.dram_tensor("wqT", (DIM, HGD), f16, kind="ExternalInput").ap()
    wkvT = nc.dram_tensor("wkvT", (DIM, 2 * HGD), f16, kind="ExternalInput").ap()
    eT = nc.dram_tensor("eT", (N, HG * K), f16, kind="ExternalInput").ap()
    fT = nc.dram_tensor("fT", (N, HG * K), f16, kind="ExternalInput").ap()
    woutT = nc.dram_tensor("woutT", (HGD, DIM), f16, kind="ExternalInput").ap()
    y = nc.dram_tensor("y", (N, DIM), f16, kind="ExternalOutput").ap()

    def hloc(h):
        # head h (0..7) -> (tile idx, partition base, free base) in the
        # packed [128, 512] projT psum/sbuf tiles
        sub = h % 4
        return h // 4, (sub % 2) * 64, (sub // 2) * 256

    st = {}

    def issue_q_quarter(g, qc):
        """One qdim-chunk of q for a future group -- PE filler work
        interleaved with other phases."""
        x_s, wq_s, q_s, psq = st["x_s"], st["wq_s"], st["q_s"], st["psq"]
        pq = psq.tile([128, 512], f32, name="pq")
        for dc in range(8):
            nc.tensor.matmul(
                pq[:],
                wq_s[:, dc * HGD + qc * 128: dc * HGD + (qc + 1) * 128],
                x_s[:, dc * N + g * TG: dc * N + (g + 1) * TG],
                start=(dc == 0), stop=(dc == 7))
        nc.scalar.copy(
            q_s[:, qc * N + g * TG: qc * N + (g + 1) * TG], pq[:])
    st["issue_q_quarter"] = issue_q_quarter

    def phase_kv(tc):
        efp, kvp = st["efp"], st["kvp"]
        pskv, pspr = st["pskv"], st["pspr"]
        x_s, wkv_s = st["x_s"], st["wkv_s"]
        projp = st["projp"]

        kprojT_ps = [pspr.tile([128, 512], f32, name=f"kprojTps{i}")
                     for i in range(2)]
        vprojT_ps = [pspr.tile([128, 512], f32, name=f"vprojTps{i}")
                     for i in range(2)]

        wq_s, wout_s = st["wq_s"], st["wout_s"]
        for tc_i in range(NTC):
            if tc_i == 12:
                for dc in range(8):
                    nc.sync.dma_start(wq_s[:, dc * HGD:(dc + 1) * HGD],
                                      wqT[dc * 128:(dc + 1) * 128, :])
            if tc_i == 16:
                for dc in range(4):
                    nc.sync.dma_start(wout_s[:, dc * DIM:(dc + 1) * DIM],
                                      woutT[dc * 128:(dc + 1) * 128, :])

            if tc_i == 0:
                ef0 = st["ef0"]
                eg = ef0[:, 0:HG * K]
                fg = ef0[:, HG * K:2 * HG * K]
            else:
                eg_t = efp.tile([128, HG * K], f16, name="eg")
                fg_t = efp.tile([128, HG * K], f16, name="fg")
                nc.sync.dma_start(eg_t[:], eT[tc_i * 128:(tc_i + 1) * 128, :])
                nc.sync.dma_start(fg_t[:], fT[tc_i * 128:(tc_i + 1) * 128, :])
                eg, fg = eg_t[:], fg_t[:]
            # x prefetch AFTER the E/F issue so it never queues ahead of
            # the stream the proj matmuls wait on
            g2 = tc_i // 4 + 2
            if g2 < NGRP:
                for dc in (2 * (tc_i % 4), 2 * (tc_i % 4) + 1):
                    nc.sync.dma_start(
                        x_s[:, dc * N + g2 * TG: dc * N + (g2 + 1) * TG],
                        xT[dc * 128:(dc + 1) * 128, g2 * TG:(g2 + 1) * TG])
            pk = pskv.tile([128, 512], f32, name="pk")
            pv = pskv.tile([128, 512], f32, name="pv")
            for dc in range(8):
                xc = x_s[:, dc * N + tc_i * 128: dc * N + (tc_i + 1) * 128]
                nc.tensor.matmul(pk[:], xc,
                                 wkv_s[:, dc * 1024: dc * 1024 + 512],
                                 start=(dc == 0), stop=(dc == 7))
            kvt = kvp.tile([128, 1024], f16)
            nc.scalar.copy(kvt[:, 0:512], pk[:])
            for dc in range(8):
                xc = x_s[:, dc * N + tc_i * 128: dc * N + (tc_i + 1) * 128]
                nc.tensor.matmul(pv[:], xc,
                                 wkv_s[:, dc * 1024 + 512: dc * 1024 + 1024],
                                 start=(dc == 0), stop=(dc == 7))
            nc.scalar.copy(kvt[:, 512:1024], pv[:])
            # accumulate k_projT / v_projT over token chunks.
            # psum zero regions are 2KB per partition row: the two heads
            # sharing (tile, partition half) share one accumulation
            # group -> start on fb==0 head, stop on fb==256 head.
            for h in range(HG):
                i, pb, fb = hloc(h)
                nc.tensor.matmul(
                    kprojT_ps[i][pb:pb + 64, fb:fb + 256],
                    kvt[:, h * 64:(h + 1) * 64],
                    eg[:, h * K:(h + 1) * K],
                    start=(tc_i == 0 and fb == 0),
                    stop=(tc_i == NTC - 1 and fb == 256),
                    skip_group_check=True)
            for h in range(HG):
                i, pb, fb = hloc(h)
                nc.tensor.matmul(
                    vprojT_ps[i][pb:pb + 64, fb:fb + 256],
                    kvt[:, 512 + h * 64: 512 + (h + 1) * 64],
                    fg[:, h * K:(h + 1) * K],
                    start=(tc_i == 0 and fb == 0),
                    stop=(tc_i == NTC - 1 and fb == 256),
                    skip_group_check=True)

        kprojT_sb = [projp.tile([128, 512], f16, name=f"kprojT{i}")
                     for i in range(2)]
        vprojT_sb = [projp.tile([128, 512], f16, name=f"vprojT{i}")
                     for i in range(2)]
        for i in range(2):
            nc.scalar.copy(kprojT_sb[i][:], kprojT_ps[i][:])
            nc.scalar.copy(vprojT_sb[i][:], vprojT_ps[i][:])
        st["kprojT_sb"] = kprojT_sb
        st["vprojT_sb"] = vprojT_sb

    def phase_vp(tc):
        ident, projp = st["ident"], st["projp"]
        vprojT_sb, psvp = st["vprojT_sb"], st["psvp"]
        # vproj_sb[kc]: [128 K-chunk, 8 heads * 64 hd] fp16
        vproj_sb = [projp.tile([128, 512], f16, name=f"vproj{i}")
                    for i in range(2)]
        pvp = [psvp.tile([128, 512], f16, name=f"pvp{kc}") for kc in range(2)]
        for i in range(2):
            for c in range(4):
                h0 = 4 * i + (c // 2) * 2
                nc.tensor.transpose(
                    pvp[c % 2][:, h0 * 64: h0 * 64 + 128],
                    vprojT_sb[i][:, c * 128:(c + 1) * 128],
                    ident[:])
        for kc in range(2):
            nc.scalar.copy(vproj_sb[kc][:], pvp[kc][:])
        st["vproj_sb"] = vproj_sb

    def phase_attn(tc):
        ident = st["ident"]
        kprojT_sb, vproj_sb = st["kprojT_sb"], st["vproj_sb"]
        q_s, wout_s = st["q_s"], st["wout_s"]
        pep, at2p, hgp, ysbp, vecp = (st["pep"], st["at2p"], st["hgp"],
                                      st["ysbp"], st["vecp"])
        pss, psat, psho, psy = st["pss"], st["psat"], st["psho"], st["psy"]

        def issue_scores_softmax(g, hp):
            """Scores + softmax + transpose for one head pair; the
            normalized fp16 attn lands transposed in at2
            [128 K-part, (hs,kc) blocks of 512, token]."""
            at2 = at2p.tile([128, 2048], f16, name="at2")
            at2_3d = at2[:].rearrange("p (j c) -> p j c", c=512)
            den8 = vecp.tile([128, 8], f32, name="den8")
            r8 = vecp.tile([128, 8], f32, name="r8")
            pes = []
            for half in range(2):
                # one psum bank per head (a bank must only ever be
                # written from one PE row position -- mixing faults HW)
                pe2 = [pep.tile([128, 512], f16, name=f"pe{j}",
                                tag="pe") for j in range(2)]
                for hs in range(2):
                    h = hp * 2 + hs
                    i, pb, fb = hloc(h)
                    qc = h // 2
                    ps = pss.tile([128, 512], f32, name="ps")
                    for t2 in range(2):
                        t = half * 2 + t2
                        nc.tensor.matmul(
                            ps[:, t2 * 256:(t2 + 1) * 256],
                            q_s[pb:pb + 64,
                                qc * N + g * TG + t * 128:
                                qc * N + g * TG + (t + 1) * 128],
                            kprojT_sb[i][pb:pb + 64, fb:fb + 256],
                            start=True, stop=True)
                    nm = vecp.tile([128, 2], f32, name="nm")
                    nc.vector.tensor_reduce(
                        nm[:], ps[:].rearrange("p (c k) -> p c k", k=256),
                        axis=AX, op=MAX, negate=True)
                    for t2 in range(2):
                        nc.scalar.activation(
                            pe2[t2][:, hs * 256:(hs + 1) * 256],
                            ps[:, t2 * 256:(t2 + 1) * 256],
                            EXP, bias=nm[:, t2:t2 + 1])
                pes.extend(pe2)
            # den per (tile t, head hs): one 3D reduce per exp tile
            for t in range(4):
                nc.vector.tensor_reduce(
                    den8[:, t * 2:(t + 1) * 2],
                    pes[t][:].rearrange("p (c k) -> p c k", k=256),
                    axis=AX, op=ADD)
            nc.vector.reciprocal(r8[:], den8[:])
            for t in range(4):
                for hs in range(2):
                    nc.vector.tensor_scalar_mul(
                        pes[t][:, hs * 256:(hs + 1) * 256],
                        pes[t][:, hs * 256:(hs + 1) * 256],
                        r8[:, t * 2 + hs: t * 2 + hs + 1])
            # transpose: token-half 0 via DMA XBAR (SP), half 1 via PE
            for t2 in range(2):
                nc.sync.dma_start_transpose(
                    at2_3d[:, :, t2 * 128:(t2 + 1) * 128], pes[t2][:])
            pat = psat.tile([128, 1024], f16, name="pat")
            for t2 in range(2):
                for hs in range(2):
                    for kc in range(2):
                        nc.tensor.matmul(
                            pat[:, (hs * 2 + kc) * 256 + t2 * 128:
                                (hs * 2 + kc) * 256 + (t2 + 1) * 128],
                            pes[2 + t2][:, hs * 256 + kc * 128:
                                        hs * 256 + (kc + 1) * 128],
                            ident[:],
                            is_transpose=True, start=True, stop=True)
            nc.scalar.copy(
                at2_3d[:, :, 256:512],
                pat[:].rearrange("p (j c) -> p j c", c=256))
            return at2

        def issue_av(g, hp, at2, hgt):
            """AV matmuls for a pair whose transposed attn is in at2."""
            pho = psho.tile([128, 512], f32, name="pho")
            for hs in range(2):
                h = hp * 2 + hs
                for kc in range(2):
                    nc.tensor.matmul(
                        pho[hs * 64:(hs + 1) * 64, :],
                        vproj_sb[kc][:, h * 64:(h + 1) * 64],
                        at2[:, (hs * 2 + kc) * 512:(hs * 2 + kc + 1) * 512],
                        start=(kc == 0), stop=(kc == 1),
                        skip_group_check=True)
            nc.scalar.copy(hgt[hp][:], pho[:])

        issue_q_quarter = st["issue_q_quarter"]

        def issue_y(g, hgt):
            """Fused output projection for a finished token group,
            ec-blocked so the ec0 psum bank recycles during ec1's
            matmuls; copies split ACT(ec0)/DVE(ec1)."""
            for t in range(4):
                ysb = ysbp.tile([128, 1024], f16, name="ysb")
                for ec in range(2):
                    py = psy.tile([128, 512], f32, name="py")
                    for hp in range(4):
                        nc.tensor.matmul(
                            py[:],
                            hgt[hp][:, t * 128:(t + 1) * 128],
                            wout_s[:, hp * DIM + ec * 512:
                                   hp * DIM + (ec + 1) * 512],
                            start=(hp == 0), stop=(hp == 3))
                    if ec == 0:
                        nc.scalar.copy(ysb[:, 0:512], py[:])
                    else:
                        nc.vector.tensor_copy(ysb[:, 512:1024], py[:])
                nc.sync.dma_start(
                    y[(g * 4 + t) * 128:(g * 4 + t + 1) * 128, :], ysb[:])

        # software-pipelined: scores/softmax/transpose of pair i issue
        # before the AV of pair i-DEPTH, so the PE never waits on the
        # softmax chain; y projection of a group issues right after its
        # last AV.
        from collections import deque
        hgts = {}
        queue = deque()
        y_ready = None
        DEPTH = 4
        for qc in range(4):
            issue_q_quarter(0, qc)
        for qc in range(4):
            issue_q_quarter(1, qc)
        pairs = [(g, hp) for g in range(NGRP) for hp in range(4)]
        for g, hp in pairs + [(None, None)] * (DEPTH + 1):
            if g is not None:
                if hp == 0:
                    hgts[g] = [hgp.tile([128, TG], f16, name=f"hgt{i}")
                               for i in range(4)]
                queue.append((g, hp, issue_scores_softmax(g, hp)))
                if g + 2 < NGRP:
                    issue_q_quarter(g + 2, hp)
            if len(queue) > DEPTH or (g is None and queue):
                pg, php, pat2 = queue.popleft()
                issue_av(pg, php, pat2, hgts[pg])
                if php == 3:
                    y_ready = pg
                    continue
            if y_ready is not None:
                issue_y(y_ready, hgts.pop(y_ready))
                y_ready = None


    with tile.TileContext(nc) as tc:
        with (
            tc.tile_pool(name="const", bufs=1) as constp,
            tc.tile_pool(name="persist", bufs=1) as persistp,
            tc.tile_pool(name="proj_sb", bufs=1) as projp,
        ):
            ident = constp.tile([128, 128], f16)
            make_identity(nc, ident[:])
            st["ident"] = ident
            st["projp"] = projp

            x_s = persistp.tile([128, 8 * N], f16, name="x_s")
            wq_s = persistp.tile([128, 8 * HGD], f16, name="wq_s")
            wkv_s = persistp.tile([128, 8 * 2 * HGD], f16, name="wkv_s")
            wout_s = persistp.tile([128, 4 * DIM], f16, name="wout_s")
            q_s = persistp.tile([128, 4 * N], f16, name="q_s")
            st.update(x_s=x_s, wq_s=wq_s, wkv_s=wkv_s, wout_s=wout_s, q_s=q_s)

            def load_x_group(g):
                for dc in range(8):
                    nc.sync.dma_start(
                        x_s[:, dc * N + g * TG: dc * N + (g + 1) * TG],
                        xT[dc * 128:(dc + 1) * 128, g * TG:(g + 1) * TG])

            # startup: wkv-k split across the SP and ACT DGE queues and
            # x chunk 0 on ACT, so the first chunk's matmuls wait on two
            # parallel streams instead of one
            for dc in range(4):
                nc.sync.dma_start(wkv_s[:, dc * 2 * HGD: dc * 2 * HGD + HGD],
                                  wkvT[dc * 128:(dc + 1) * 128, 0:HGD])
            for dc in range(4, 8):
                nc.scalar.dma_start(wkv_s[:, dc * 2 * HGD: dc * 2 * HGD + HGD],
                                    wkvT[dc * 128:(dc + 1) * 128, 0:HGD])
            for dc in range(8):
                nc.scalar.dma_start(x_s[:, dc * N: dc * N + 128],
                                    xT[dc * 128:(dc + 1) * 128, 0:128])
            for dc in range(8):
                nc.sync.dma_start(x_s[:, dc * N + 128: dc * N + TG],
                                  xT[dc * 128:(dc + 1) * 128, 128:TG])
            st["ef0"] = ef0 = projp.tile([128, 2 * HG * K], f16, name="ef0")
            nc.scalar.dma_start(ef0[:, 0:HG * K], eT[0:128, :])
            for dc in range(8):
                nc.sync.dma_start(
                    wkv_s[:, dc * 2 * HGD + HGD:(dc + 1) * 2 * HGD],
                    wkvT[dc * 128:(dc + 1) * 128, HGD:2 * HGD])
            nc.scalar.dma_start(ef0[:, HG * K:2 * HG * K], fT[0:128, :])
            load_x_group(1)
            st["load_x_group"] = load_x_group

            psq_ctx = tc.tile_pool(name="ps_q", bufs=1, space="PSUM")
            st["psq"] = psq_ctx.__enter__()
            with (
                tc.tile_pool(name="ef", bufs=3) as efp,
                tc.tile_pool(name="kv", bufs=3) as kvp,
                tc.tile_pool(name="ps_kv", bufs=1, space="PSUM") as pskv,
                tc.tile_pool(name="ps_proj", bufs=1, space="PSUM") as pspr,
            ):
                st.update(efp=efp, kvp=kvp, pskv=pskv, pspr=pspr)
                phase_kv(tc)

            if level >= 2:
                with tc.tile_pool(name="ps_vp", bufs=1, space="PSUM") as psvp:
                    st["psvp"] = psvp
                    phase_vp(tc)

            if level >= 4:
              with (
                tc.tile_pool(name="pe", bufs=16) as pep,
                tc.tile_pool(name="at2", bufs=5) as at2p,
                tc.tile_pool(name="hgt", bufs=3) as hgp,
                tc.tile_pool(name="ysb", bufs=3) as ysbp,
                tc.tile_pool(name="vec", bufs=16) as vecp,
                tc.tile_pool(name="ps_s", bufs=2, space="PSUM") as pss,
                tc.tile_pool(name="ps_at", bufs=2, space="PSUM") as psat,
                tc.tile_pool(name="ps_ho", bufs=1, space="PSUM") as psho,
                tc.tile_pool(name="ps_y", bufs=2, space="PSUM") as psy,
            ):
                st.update(pep=pep, at2p=at2p, hgp=hgp, ysbp=ysbp, vecp=vecp,
                          pss=pss, psat=psat, psho=psho, psy=psy)
                phase_attn(tc)

            psq_ctx.__exit__(None, None, None)

    if level < 4:
        with tile.TileContext(nc) as tc2:
            with tc2.tile_pool(name="dummy", bufs=1) as dp:
                zt = dp.tile([128, DIM], f16)
                nc.gpsimd.memset(zt[:], 0.0)
                for gb in range(NTC):
                    nc.sync.dma_start(y[gb * 128:(gb + 1) * 128, :], zt[:])

    nc.compile()
    return nc


def _prep_inputs(x, Wqkv, E, F, Wout):
    """Build the 8 per-core input dicts (host-side slicing/transposes)."""
    f16 = np.float16
    ins = []
    per_hg = {}
    for hg in range(2):
        r = hg * HGD
        wqT = np.ascontiguousarray(Wqkv[r:r + HGD, :].T, dtype=f16)
        wk = Wqkv[DIM + r: DIM + r + HGD, :]
        wv = Wqkv[2 * DIM + r: 2 * DIM + r + HGD, :]
        wkvT = np.concatenate([wk.T, wv.T], axis=1).astype(f16)
        # E pre-scaled by SCALE so scores come out of the PE pre-scaled
        eT = np.ascontiguousarray(
            E[hg * HG:(hg + 1) * HG].transpose(2, 0, 1).reshape(N, HG * K)
            * SCALE, dtype=f16)
        fT = np.ascontiguousarray(
            F[hg * HG:(hg + 1) * HG].transpose(2, 0, 1).reshape(N, HG * K),
            dtype=f16)
        woutT = np.ascontiguousarray(Wout[:, r:r + HGD].T, dtype=f16)
        per_hg[hg] = (wqT, wkvT, eT, fT, woutT)
    xTs = [np.ascontiguousarray(x[b].T, dtype=f16) for b in range(B)]
    for c in range(NCORES):
        b, hg = c // 2, c % 2
        wqT, wkvT, eT, fT, woutT = per_hg[hg]
        ins.append({"xT": xTs[b], "wqT": wqT, "wkvT": wkvT,
                    "eT": eT, "fT": fT, "woutT": woutT})
    return ins


def kernel(x, Wqkv, E, F, Wout, bout):
    from concourse.bass_utils import run_bass_kernel_spmd

    x = np.asarray(x, dtype=np.float32)
    Wqkv = np.asarray(Wqkv, dtype=np.float32)
    E = np.asarray(E, dtype=np.float32)
    F = np.asarray(F, dtype=np.float32)
    Wout = np.asarray(Wout, dtype=np.float32)
    bout = np.asarray(bout, dtype=np.float32)

    if "nc" not in _cache:
        _cache["nc"] = _build()
    nc = _cache["nc"]

    in_maps = _prep_inputs(x, Wqkv, E, F, Wout)
    res = run_bass_kernel_spmd(nc, in_maps, core_ids=list(range(NCORES)))
    out = np.empty((B, N, DIM), dtype=np.float32)
    for b in range(B):
        out[b] = (res.results[2 * b]["y"].astype(np.float32)
                  + res.results[2 * b + 1]["y"].astype(np.float32) + bout)
    return out


# revision 8
# speedup vs baseline: 2.7990x; 1.0082x over previous
"""Linformer self-attention Trainium2 kernel (fp16 PE pipeline).

Problem (hardcoded): B=4, N=4096, DIM=1024, H=16, K=256, HD=64, fp32 I/O.
  qkv = x @ Wqkv.T; q,k,v split into 16 heads of 64
  k_proj = E @ k, v_proj = F @ v  (per head, contract over tokens)
  out = softmax(q @ k_proj.T / 8) @ v_proj
  y = out @ Wout.T + bout

Sharding: 8 cores = (batch b = c//2) x (head-group hg = c%2, 8 heads each).
Each core computes a (4096, 1024) fp16 partial of y for its batch; host
sums hg=0 + hg=1 partials in fp32 and adds bout. No collectives.

All matmuls run in fp16. E is pre-scaled by 1/8 on the host so scores
come out of the PE pre-scaled and the negated max feeds exp's bias
directly. Per core:
  Phase A: as before (k,v per 128-token chunk contracted with streamed
    E/F into k_projT/v_projT psum accumulators). Startup split across
    the SP and ACT DGE queues so the first matmul starts earlier.
  Phase C (software-pipelined over 32 head-pair slots), engine-balanced:
    scores (PE) -> batched DVE max -> ACT exp (bias = negated max, no
    accumulator) -> DVE den sums (one 3D reduce per exp tile into a
    shared [128,8] tile) -> one DVE reciprocal -> gpsimd normalize.
    The fp16 attn transpose is split: token-half 0 goes through the DMA
    XBAR (dma_start_transpose on SP), token-half 1 through PE transpose
    matmuls + one DVE psum copy; both land in one at2 [128 K, 2048]
    tile per pair. AV issues DEPTH slots later. The output projection
    runs ec-blocked (4 matmuls + copy per 512-wide half) so the psum
    bank for ec0 recycles while ec1's matmuls run; y copies split
    ACT(ec0)/DVE(ec1); q copies on ACT.
"""

import numpy as np

B, N, DIM, H, K = 4, 4096, 1024, 16, 256
HD = DIM // H
SCALE = 1.0 / 8.0
HG = H // 2          # 8 heads per core
HGD = HG * HD        # 512 head dims per core
NCORES = 8
TG = 512             # token group
NTC = N // 128       # 32 token chunks
NGRP = N // TG       # 8 token groups

_cache = {}


def _build(level=4):
    import concourse.mybir as mybir
    import concourse.tile as tile
    from concourse import bacc
    from concourse.masks import make_identity

    f32 = mybir.dt.float32
    f16 = mybir.dt.float16
    AX = mybir.AxisListType.X
    MAX = mybir.AluOpType.max
    ADD = mybir.AluOpType.add
    EXP = mybir.ActivationFunctionType.Exp

    nc = bacc.Bacc("TRN2", target_bir_lowering=False, debug=False,
                   enable_asserts=False)

    xT = nc.dram_tensor("xT", (DIM, N), f16, kind="ExternalInput").ap()
    wqT = nc.dram_tensor("wqT", (DIM, HGD), f16, kind="ExternalInput").ap()
    wkvT = nc.dram_tensor("wkvT", (DIM, 2 * HGD), f16, kind="ExternalInput").ap()
    eT = nc.dram_tensor("eT", (N, HG * K), f16, kind="ExternalInput").ap()
    fT = nc.dram_tensor("fT", (N, HG * K), f16, kind="ExternalInput").ap()
    woutT = nc.dram_tensor("woutT", (HGD, DIM), f16, kind="ExternalInput").ap()
    y = nc.dram_tensor("y", (N, DIM), f16, kind="ExternalOutput").ap()

    def hloc(h):
        # head h (0..7) -> (tile idx, partition base, free base) in the
        # packed [128, 512] projT psum/sbuf tiles
        sub = h % 4
        return h // 4, (sub % 2) * 64, (sub // 2) * 256

    st = {}

    def issue_q_quarter(g, qc):
        """One qdim-chunk of q for a future group -- PE filler work
        interleaved with other phases."""
        x_s, wq_s, q_s, psq = st["x_s"], st["wq_s"], st["q_s"], st["psq"]
        pq = psq.tile([128, 512], f32, name="pq")
        for dc in range(8):
            nc.tensor.matmul(
                pq[:],
                wq_s[:, dc * HGD + qc * 128: dc * HGD + (qc + 1) * 128],
                x_s[:, dc * N + g * TG: dc * N + (g + 1) * TG],
                start=(dc == 0), stop=(dc == 7))
        nc.scalar.copy(
            q_s[:, qc * N + g * TG: qc * N + (g + 1) * TG], pq[:])
    st["issue_q_quarter"] = issue_q_quarter

    def phase_kv(tc):
        efp, kvp = st["efp"], st["kvp"]
        pskv, pspr = st["pskv"], st["pspr"]
        x_s, wkv_s = st["x_s"], st["wkv_s"]
        projp = st["projp"]

        kprojT_ps = [pspr.tile([128, 512], f32, name=f"kprojTps{i}")
                     for i in range(2)]
        vprojT_ps = [pspr.tile([128, 512], f32, name=f"vprojTps{i}")
                     for i in range(2)]

        wq_s, wout_s = st["wq_s"], st["wout_s"]
        for tc_i in range(NTC):
            if tc_i == 12:
                for dc in range(8):
                    nc.sync.dma_start(wq_s[:, dc * HGD:(dc + 1) * HGD],
                                      wqT[dc * 128:(dc + 1) * 128, :])
            if tc_i == 16:
                for dc in range(4):
                    nc.sync.dma_start(wout_s[:, dc * DIM:(dc + 1) * DIM],
                                      woutT[dc * 128:(dc + 1) * 128, :])

            if tc_i == 0:
                ef0 = st["ef0"]
                eg = ef0[:, 0:HG * K]
                fg = ef0[:, HG * K:2 * HG * K]
            else:
                eg_t = efp.tile([128, HG * K], f16, name="eg")
                fg_t = efp.tile([128, HG * K], f16, name="fg")
                nc.sync.dma_start(eg_t[:], eT[tc_i * 128:(tc_i + 1) * 128, :])
                nc.sync.dma_start(fg_t[:], fT[tc_i * 128:(tc_i + 1) * 128, :])
                eg, fg = eg_t[:], fg_t[:]
            # x prefetch AFTER the E/F issue so it never queues ahead of
            # the stream the proj matmuls wait on
            g2 = tc_i // 4 + 2
            if g2 < NGRP:
                for dc in (2 * (tc_i % 4), 2 * (tc_i % 4) + 1):
                    nc.sync.dma_start(
                        x_s[:, dc * N + g2 * TG: dc * N + (g2 + 1) * TG],
                        xT[dc * 128:(dc + 1) * 128, g2 * TG:(g2 + 1) * TG])
            pk = pskv.tile([128, 512], f32, name="pk")
            pv = pskv.tile([128, 512], f32, name="pv")
            for dc in range(8):
                xc = x_s[:, dc * N + tc_i * 128: dc * N + (tc_i + 1) * 128]
                nc.tensor.matmul(pk[:], xc,
                                 wkv_s[:, dc * 1024: dc * 1024 + 512],
                                 start=(dc == 0), stop=(dc == 7))
            kvt = kvp.tile([128, 1024], f16)
            nc.scalar.copy(kvt[:, 0:512], pk[:])
            for dc in range(8):
                xc = x_s[:, dc * N + tc_i * 128: dc * N + (tc_i + 1) * 128]
                nc.tensor.matmul(pv[:], xc,
                                 wkv_s[:, dc * 1024 + 512: dc * 1024 + 1024],
                                 start=(dc == 0), stop=(dc == 7))
            nc.scalar.copy(kvt[:, 512:1024], pv[:])
            # accumulate k_projT / v_projT over token chunks.
            # psum zero regions are 2KB per partition row: the two heads
            # sharing (tile, partition half) share one accumulation
            # group -> start on fb==0 head, stop on fb==256 head.
            for h in range(HG):
                i, pb, fb = hloc(h)
                nc.tensor.matmul(
                    kprojT_ps[i][pb:pb + 64, fb:fb + 256],
                    kvt[:, h * 64:(h + 1) * 64],
                    eg[:, h * K:(h + 1) * K],
                    start=(tc_i == 0 and fb == 0),
                    stop=(tc_i == NTC - 1 and fb == 256),
                    skip_group_check=True)
            for h in range(HG):
                i, pb, fb = hloc(h)
                nc.tensor.matmul(
                    vprojT_ps[i][pb:pb + 64, fb:fb + 256],
                    kvt[:, 512 + h * 64: 512 + (h + 1) * 64],
                    fg[:, h * K:(h + 1) * K],
                    start=(tc_i == 0 and fb == 0),
                    stop=(tc_i == NTC - 1 and fb == 256),
                    skip_group_check=True)

        kprojT_sb = [projp.tile([128, 512], f16, name=f"kprojT{i}")
                     for i in range(2)]
        vprojT_sb = [projp.tile([128, 512], f16, name=f"vprojT{i}")
                     for i in range(2)]
        for i in range(2):
            nc.scalar.copy(kprojT_sb[i][:], kprojT_ps[i][:])
            nc.scalar.copy(vprojT_sb[i][:], vprojT_ps[i][:])
        st["kprojT_sb"] = kprojT_sb
        st["vprojT_sb"] = vprojT_sb

    def phase_vp(tc):
        ident, projp = st["ident"], st["projp"]
        vprojT_sb, psvp = st["vprojT_sb"], st["psvp"]
        # vproj_sb[kc]: [128 K-chunk, 8 heads * 64 hd] fp16
        vproj_sb = [projp.tile([128, 512], f16, name=f"vproj{i}")
                    for i in range(2)]
        pvp = [psvp.tile([128, 512], f16, name=f"pvp{kc}") for kc in range(2)]
        for i in range(2):
            for c in range(4):
                h0 = 4 * i + (c // 2) * 2
                nc.tensor.transpose(
                    pvp[c % 2][:, h0 * 64: h0 * 64 + 128],
                    vprojT_sb[i][:, c * 128:(c + 1) * 128],
                    ident[:])
        for kc in range(2):
            nc.scalar.copy(vproj_sb[kc][:], pvp[kc][:])
        st["vproj_sb"] = vproj_sb

    def phase_attn(tc):
        ident = st["ident"]
        kprojT_sb, vproj_sb = st["kprojT_sb"], st["vproj_sb"]
        q_s, wout_s = st["q_s"], st["wout_s"]
        pep, at2p, hgp, ysbp, vecp = (st["pep"], st["at2p"], st["hgp"],
                                      st["ysbp"], st["vecp"])
        pss, psat, psho, psy = st["pss"], st["psat"], st["psho"], st["psy"]

        def issue_scores_softmax(g, hp):
            """Scores + softmax + transpose for one head pair; the
            normalized fp16 attn lands transposed in at2
            [128 K-part, (hs,kc) blocks of 512, token]."""
            at2 = at2p.tile([128, 2048], f16, name="at2")
            at2_3d = at2[:].rearrange("p (j c) -> p j c", c=512)
            den8 = vecp.tile([128, 8], f32, name="den8")
            r8 = vecp.tile([128, 8], f32, name="r8")
            pes = []
            for half in range(2):
                # one psum bank per head (a bank must only ever be
                # written from one PE row position -- mixing faults HW)
                pe2 = [pep.tile([128, 512], f16, name=f"pe{j}",
                                tag="pe") for j in range(2)]
                for hs in range(2):
                    h = hp * 2 + hs
                    i, pb, fb = hloc(h)
                    qc = h // 2
                    ps = pss.tile([128, 512], f32, name="ps")
                    for t2 in range(2):
                        t = half * 2 + t2
                        nc.tensor.matmul(
                            ps[:, t2 * 256:(t2 + 1) * 256],
                            q_s[pb:pb + 64,
                                qc * N + g * TG + t * 128:
                                qc * N + g * TG + (t + 1) * 128],
                            kprojT_sb[i][pb:pb + 64, fb:fb + 256],
                            start=True, stop=True)
                    nm = vecp.tile([128, 2], f32, name="nm")
                    nc.vector.tensor_reduce(
                        nm[:], ps[:].rearrange("p (c k) -> p c k", k=256),
                        axis=AX, op=MAX, negate=True)
                    for t2 in range(2):
                        nc.scalar.activation(
                            pe2[t2][:, hs * 256:(hs + 1) * 256],
                            ps[:, t2 * 256:(t2 + 1) * 256],
                            EXP, bias=nm[:, t2:t2 + 1])
                pes.extend(pe2)
            # den per (tile t, head hs): one 3D reduce per exp tile
            for t in range(4):
                nc.vector.tensor_reduce(
                    den8[:, t * 2:(t + 1) * 2],
                    pes[t][:].rearrange("p (c k) -> p c k", k=256),
                    axis=AX, op=ADD)
            nc.vector.reciprocal(r8[:], den8[:])
            for t in range(4):
                for hs in range(2):
                    nc.vector.tensor_scalar_mul(
                        pes[t][:, hs * 256:(hs + 1) * 256],
                        pes[t][:, hs * 256:(hs + 1) * 256],
                        r8[:, t * 2 + hs: t * 2 + hs + 1])
            # transpose via PE into two psum tiles, copied out on ACT
            for half in range(2):
                pat = psat.tile([128, 1024], f16, name="pat")
                for t2 in range(2):
                    for hs in range(2):
                        for kc in range(2):
                            nc.tensor.matmul(
                                pat[:, (hs * 2 + kc) * 256 + t2 * 128:
                                    (hs * 2 + kc) * 256 + (t2 + 1) * 128],
                                pes[half * 2 + t2][:, hs * 256 + kc * 128:
                                                   hs * 256 + (kc + 1) * 128],
                                ident[:],
                                is_transpose=True, start=True, stop=True)
                nc.scalar.copy(
                    at2_3d[:, :, half * 256:(half + 1) * 256],
                    pat[:].rearrange("p (j c) -> p j c", c=256))
            return at2

        def issue_av(g, hp, at2, hgt):
            """AV matmuls for a pair whose transposed attn is in at2."""
            pho = psho.tile([128, 512], f32, name="pho")
            for hs in range(2):
                h = hp * 2 + hs
                for kc in range(2):
                    nc.tensor.matmul(
                        pho[hs * 64:(hs + 1) * 64, :],
                        vproj_sb[kc][:, h * 64:(h + 1) * 64],
                        at2[:, (hs * 2 + kc) * 512:(hs * 2 + kc + 1) * 512],
                        start=(kc == 0), stop=(kc == 1),
                        skip_group_check=True)
            nc.vector.tensor_copy(hgt[hp][:], pho[:])

        issue_q_quarter = st["issue_q_quarter"]

        def issue_y(g, hgt):
            """Fused output projection for a finished token group,
            ec-blocked so the ec0 psum bank recycles during ec1's
            matmuls; copies split ACT(ec0)/DVE(ec1)."""
            for t in range(4):
                ysb = ysbp.tile([128, 1024], f16, name="ysb")
                for ec in range(2):
                    py = psy.tile([128, 512], f32, name="py")
                    for hp in range(4):
                        nc.tensor.matmul(
                            py[:],
                            hgt[hp][:, t * 128:(t + 1) * 128],
                            wout_s[:, hp * DIM + ec * 512:
                                   hp * DIM + (ec + 1) * 512],
                            start=(hp == 0), stop=(hp == 3))
                    if ec == 0:
                        nc.scalar.copy(ysb[:, 0:512], py[:])
                    else:
                        nc.vector.tensor_copy(ysb[:, 512:1024], py[:])
                nc.sync.dma_start(
                    y[(g * 4 + t) * 128:(g * 4 + t + 1) * 128, :], ysb[:])

        # software-pipelined: scores/softmax/transpose of pair i issue
        # before the AV of pair i-DEPTH, so the PE never waits on the
        # softmax chain; y projection of a group issues right after its
        # last AV.
        from collections import deque
        hgts = {}
        queue = deque()
        y_ready = None
        DEPTH = 4
        for qc in range(4):
            issue_q_quarter(0, qc)
        for qc in range(4):
            issue_q_quarter(1, qc)
        pairs = [(g, hp) for g in range(NGRP) for hp in range(4)]
        for g, hp in pairs + [(None, None)] * (DEPTH + 1):
            if g is not None:
                if hp == 0:
                    hgts[g] = [hgp.tile([128, TG], f16, name=f"hgt{i}")
                               for i in range(4)]
                queue.append((g, hp, issue_scores_softmax(g, hp)))
                if g + 2 < NGRP:
                    issue_q_quarter(g + 2, hp)
            if len(queue) > DEPTH or (g is None and queue):
                pg, php, pat2 = queue.popleft()
                issue_av(pg, php, pat2, hgts[pg])
                if php == 3:
                    y_ready = pg
                    continue
            if y_ready is not None:
                issue_y(y_ready, hgts.pop(y_ready))
                y_ready = None


    with tile.TileContext(nc) as tc:
        with (
            tc.tile_pool(name="const", bufs=1) as constp,
            tc.tile_pool(name="persist", bufs=1) as persistp,
            tc.tile_pool(name="proj_sb", bufs=1) as projp,
        ):
            ident = constp.tile([128, 128], f16)
            make_identity(nc, ident[:])
            st["ident"] = ident
            st["projp"] = projp

            x_s = persistp.tile([128, 8 * N], f16, name="x_s")
            wq_s = persistp.tile([128, 8 * HGD], f16, name="wq_s")
            wkv_s = persistp.tile([128, 8 * 2 * HGD], f16, name="wkv_s")
            wout_s = persistp.tile([128, 4 * DIM], f16, name="wout_s")
            q_s = persistp.tile([128, 4 * N], f16, name="q_s")
            st.update(x_s=x_s, wq_s=wq_s, wkv_s=wkv_s, wout_s=wout_s, q_s=q_s)

            def load_x_group(g):
                for dc in range(8):
                    nc.sync.dma_start(
                        x_s[:, dc * N + g * TG: dc * N + (g + 1) * TG],
                        xT[dc * 128:(dc + 1) * 128, g * TG:(g + 1) * TG])

            # startup: wkv-k split across the SP and ACT DGE queues and
            # x chunk 0 on ACT, so the first chunk's matmuls wait on two
            # parallel streams instead of one
            for dc in range(4):
                nc.sync.dma_start(wkv_s[:, dc * 2 * HGD: dc * 2 * HGD + HGD],
                                  wkvT[dc * 128:(dc + 1) * 128, 0:HGD])
            for dc in range(4, 8):
                nc.scalar.dma_start(wkv_s[:, dc * 2 * HGD: dc * 2 * HGD + HGD],
                                    wkvT[dc * 128:(dc + 1) * 128, 0:HGD])
            for dc in range(8):
                nc.scalar.dma_start(x_s[:, dc * N: dc * N + 128],
                                    xT[dc * 128:(dc + 1) * 128, 0:128])
            for dc in range(8):
                nc.sync.dma_start(x_s[:, dc * N + 128: dc * N + TG],
                                  xT[dc * 128:(dc + 1) * 128, 128:TG])
            st["ef0"] = ef0 = projp.tile([128, 2 * HG * K], f16, name="ef0")
            nc.scalar.dma_start(ef0[:, 0:HG * K], eT[0:128, :])
            for dc in range(8):
                nc.sync.dma_start(
                    wkv_s[:, dc * 2 * HGD + HGD:(dc + 1) * 2 * HGD],
                    wkvT[dc * 128:(dc + 1) * 128, HGD:2 * HGD])
            nc.scalar.dma_start(ef0[:, HG * K:2 * HG * K], fT[0:128, :])
            load_x_group(1)
            st["load_x_group"] = load_x_group

            psq_ctx = tc.tile_pool(name="ps_q", bufs=1, space="PSUM")
            st["psq"] = psq_ctx.__enter__()
            with (
                tc.tile_pool(name="ef", bufs=3) as efp,
                tc.tile_pool(name="kv", bufs=3) as kvp,
                tc.tile_pool(name="ps_kv", bufs=1, space="PSUM") as pskv,
                tc.tile_pool(name="ps_proj", bufs=1, space="PSUM") as pspr,
            ):
                st.update(efp=efp, kvp=kvp, pskv=pskv, pspr=pspr)
                phase_kv(tc)

            if level >= 2:
                with tc.tile_pool(name="ps_vp", bufs=1, space="PSUM") as psvp:
                    st["psvp"] = psvp
                    phase_vp(tc)

            if level >= 4:
              with (
                tc.tile_pool(name="pe", bufs=16) as pep,
                tc.tile_pool(name="at2", bufs=5) as at2p,
                tc.tile_pool(name="hgt", bufs=3) as hgp,
                tc.tile_pool(name="ysb", bufs=3) as ysbp,
                tc.tile_pool(name="vec", bufs=16) as vecp,
                tc.tile_pool(name="ps_s", bufs=2, space="PSUM") as pss,
                tc.tile_pool(name="ps_at", bufs=2, space="PSUM") as psat,
                tc.tile_pool(name="ps_ho", bufs=1, space="PSUM") as psho,
                tc.tile_pool(name="ps_y", bufs=2, space="PSUM") as psy,
            ):
                st.update(pep=pep, at2p=at2p, hgp=hgp, ysbp=ysbp, vecp=vecp,
                          pss=pss, psat=psat, psho=psho, psy=psy)
                phase_attn(tc)

            psq_ctx.__exit__(None, None, None)

    if level < 4:
        with tile.TileContext(nc) as tc2:
            with tc2.tile_pool(name="dummy", bufs=1) as dp:
                zt = dp.tile([128, DIM], f16)
                nc.gpsimd.memset(zt[:], 0.0)
                for gb in range(NTC):
                    nc.sync.dma_start(y[gb * 128:(gb + 1) * 128, :], zt[:])

    nc.compile()
    return nc


def _prep_inputs(x, Wqkv, E, F, Wout):
    """Build the 8 per-core input dicts (host-side slicing/transposes)."""
    f16 = np.float16
    ins = []
    per_hg = {}
    for hg in range(2):
        r = hg * HGD
        wqT = np.ascontiguousarray(Wqkv[r:r + HGD, :].T, dtype=f16)
        wk = Wqkv[DIM + r: DIM + r + HGD, :]
        wv = Wqkv[2 * DIM + r: 2 * DIM + r + HGD, :]
        wkvT = np.concatenate([wk.T, wv.T], axis=1).astype(f16)
        # E pre-scaled by SCALE so scores come out of the PE pre-scaled
        eT = np.ascontiguousarray(
            E[hg * HG:(hg + 1) * HG].transpose(2, 0, 1).reshape(N, HG * K)
            * SCALE, dtype=f16)
        fT = np.ascontiguousarray(
            F[hg * HG:(hg + 1) * HG].transpose(2, 0, 1).reshape(N, HG * K),
            dtype=f16)
        woutT = np.ascontiguousarray(Wout[:, r:r + HGD].T, dtype=f16)
        per_hg[hg] = (wqT, wkvT, eT, fT, woutT)
    xTs = [np.ascontiguousarray(x[b].T, dtype=f16) for b in range(B)]
    for c in range(NCORES):
        b, hg = c // 2, c % 2
        wqT, wkvT, eT, fT, woutT = per_hg[hg]
        ins.append({"xT": xTs[b], "wqT": wqT, "wkvT": wkvT,
                    "eT": eT, "fT": fT, "woutT": woutT})
    return ins


def kernel(x, Wqkv, E, F, Wout, bout):
    from concourse.bass_utils import run_bass_kernel_spmd

    x = np.asarray(x, dtype=np.float32)
    Wqkv = np.asarray(Wqkv, dtype=np.float32)
    E = np.asarray(E, dtype=np.float32)
    F = np.asarray(F, dtype=np.float32)
    Wout = np.asarray(Wout, dtype=np.float32)
    bout = np.asarray(bout, dtype=np.float32)

    if "nc" not in _cache:
        _cache["nc"] = _build()
    nc = _cache["nc"]

    in_maps = _prep_inputs(x, Wqkv, E, F, Wout)
    res = run_bass_kernel_spmd(nc, in_maps, core_ids=list(range(NCORES)))
    out = np.empty((B, N, DIM), dtype=np.float32)
    for b in range(B):
        out[b] = (res.results[2 * b]["y"].astype(np.float32)
                  + res.results[2 * b + 1]["y"].astype(np.float32) + bout)
    return out


# revision 9
# speedup vs baseline: 3.0189x; 1.0786x over previous
"""Linformer self-attention Trainium2 kernel (fp16 PE pipeline).

Problem (hardcoded): B=4, N=4096, DIM=1024, H=16, K=256, HD=64, fp32 I/O.
  qkv = x @ Wqkv.T; q,k,v split into 16 heads of 64
  k_proj = E @ k, v_proj = F @ v  (per head, contract over tokens)
  out = softmax(q @ k_proj.T / 8) @ v_proj
  y = out @ Wout.T + bout

Sharding: 8 cores = (batch b = c//2) x (head-group hg = c%2, 8 heads each).
Each core computes a (4096, 1024) fp16 partial of y for its batch; host
sums hg=0 + hg=1 partials in fp32 and adds bout. No collectives.

All matmuls run in fp16 (1 cycle/row on the PE vs 4 for fp32; end-to-end
rel err ~5e-3). Per core:
  x resident in SBUF fp16 [128, 8*4096] (xdim-chunk major); startup DMA
    order tuned so the first k/v chunk waits on the fewest bytes.
  Phase A: k,v per 128-token chunk (tokens on partitions), immediately
    contracted with streamed E/F chunks into k_projT/v_projT psum
    accumulators (8 heads packed into 2+2 banks); x groups and the
    q/out weights stream in from inside the chunk loop; the last 8
    chunks also interleave the first two groups of q as PE filler.
  Phase A15: v_projT transposed to vproj [K on partitions] via PE.
  Phase C (software-pipelined over 32 head-pair slots): scores for a
    pair land in one psum bank per head (a psum bank must only ever be
    written from one PE row position -- mixing row positions faults the
    hardware); batched DVE max -> ACT exp (fused scale/bias, denominator
    via accum_out) -> DVE reciprocal+normalize; the fp16 attn transposes
    + AV matmuls for a pair issue DEPTH=3 slots later so the PE always
    has work while the softmax chain runs; q for group g+2 interleaves
    as filler; the output projection for a finished group issues one
    slot after its last AV and streams y out as fp16.
"""

import numpy as np

B, N, DIM, H, K = 4, 4096, 1024, 16, 256
HD = DIM // H
SCALE = 1.0 / 8.0
HG = H // 2          # 8 heads per core
HGD = HG * HD        # 512 head dims per core
NCORES = 8
TG = 512             # token group
NTC = N // 128       # 32 token chunks
NGRP = N // TG       # 8 token groups

_cache = {}


def _build(level=4):
    import concourse.mybir as mybir
    import concourse.tile as tile
    from concourse import bacc
    from concourse.masks import make_identity

    f32 = mybir.dt.float32
    f16 = mybir.dt.float16
    AX = mybir.AxisListType.X
    MAX = mybir.AluOpType.max
    EXP = mybir.ActivationFunctionType.Exp

    nc = bacc.Bacc("TRN2", target_bir_lowering=False, debug=False,
                   enable_asserts=False)

    xT = nc.dram_tensor("xT", (DIM, N), f16, kind="ExternalInput").ap()
    wqT = nc.dram_tensor("wqT", (DIM, HGD), f16, kind="ExternalInput").ap()
    wkvT = nc.dram_tensor("wkvT", (DIM, 2 * HGD), f16, kind="ExternalInput").ap()
    eT = nc.dram_tensor("eT", (N, HG * K), f16, kind="ExternalInput").ap()
    fT = nc.dram_tensor("fT", (N, HG * K), f16, kind="ExternalInput").ap()
    woutT = nc.dram_tensor("woutT", (HGD, DIM), f16, kind="ExternalInput").ap()
    y = nc.dram_tensor("y", (N, DIM), f16, kind="ExternalOutput").ap()

    def hloc(h):
        # head h (0..7) -> (tile idx, partition base, free base) in the
        # packed [128, 512] projT psum/sbuf tiles
        sub = h % 4
        return h // 4, (sub % 2) * 64, (sub // 2) * 256

    st = {}

    def issue_q_quarter(g, qc):
        """One qdim-chunk of q for a future group -- PE filler work
        interleaved with other phases."""
        x_s, wq_s, q_s, psq = st["x_s"], st["wq_s"], st["q_s"], st["psq"]
        pq = psq.tile([128, 512], f32, name="pq")
        for dc in range(8):
            nc.tensor.matmul(
                pq[:],
                wq_s[:, dc * HGD + qc * 128: dc * HGD + (qc + 1) * 128],
                x_s[:, dc * N + g * TG: dc * N + (g + 1) * TG],
                start=(dc == 0), stop=(dc == 7))
        nc.vector.tensor_copy(
            q_s[:, qc * N + g * TG: qc * N + (g + 1) * TG], pq[:])
    st["issue_q_quarter"] = issue_q_quarter

    def phase_kv(tc):
        efp, kvp = st["efp"], st["kvp"]
        pskv, pspr = st["pskv"], st["pspr"]
        x_s, wkv_s = st["x_s"], st["wkv_s"]
        projp = st["projp"]

        kprojT_ps = [pspr.tile([128, 512], f32, name=f"kprojTps{i}")
                     for i in range(2)]
        vprojT_ps = [pspr.tile([128, 512], f32, name=f"vprojTps{i}")
                     for i in range(2)]

        wq_s, wout_s = st["wq_s"], st["wout_s"]
        for tc_i in range(NTC):
            if tc_i == 12:
                for dc in range(8):
                    nc.sync.dma_start(wq_s[:, dc * HGD:(dc + 1) * HGD],
                                      wqT[dc * 128:(dc + 1) * 128, :])
            if tc_i == 16:
                for dc in range(4):
                    nc.sync.dma_start(wout_s[:, dc * DIM:(dc + 1) * DIM],
                                      woutT[dc * 128:(dc + 1) * 128, :])

            if tc_i == 0:
                ef0 = st["ef0"]
                eg = ef0[:, 0:HG * K]
                fg = ef0[:, HG * K:2 * HG * K]
            else:
                eg_t = efp.tile([128, HG * K], f16, name="eg")
                fg_t = efp.tile([128, HG * K], f16, name="fg")
                nc.sync.dma_start(eg_t[:], eT[tc_i * 128:(tc_i + 1) * 128, :])
                nc.sync.dma_start(fg_t[:], fT[tc_i * 128:(tc_i + 1) * 128, :])
                eg, fg = eg_t[:], fg_t[:]
            # x prefetch AFTER the E/F issue so it never queues ahead of
            # the stream the proj matmuls wait on
            g2 = tc_i // 4 + 2
            if g2 < NGRP:
                for dc in (2 * (tc_i % 4), 2 * (tc_i % 4) + 1):
                    nc.sync.dma_start(
                        x_s[:, dc * N + g2 * TG: dc * N + (g2 + 1) * TG],
                        xT[dc * 128:(dc + 1) * 128, g2 * TG:(g2 + 1) * TG])
            pk = pskv.tile([128, 512], f32, name="pk")
            pv = pskv.tile([128, 512], f32, name="pv")
            for dc in range(8):
                xc = x_s[:, dc * N + tc_i * 128: dc * N + (tc_i + 1) * 128]
                nc.tensor.matmul(pk[:], xc,
                                 wkv_s[:, dc * 1024: dc * 1024 + 512],
                                 start=(dc == 0), stop=(dc == 7))
            kvt = kvp.tile([128, 1024], f16)
            nc.scalar.copy(kvt[:, 0:512], pk[:])
            for dc in range(8):
                xc = x_s[:, dc * N + tc_i * 128: dc * N + (tc_i + 1) * 128]
                nc.tensor.matmul(pv[:], xc,
                                 wkv_s[:, dc * 1024 + 512: dc * 1024 + 1024],
                                 start=(dc == 0), stop=(dc == 7))
            nc.scalar.copy(kvt[:, 512:1024], pv[:])
            # accumulate k_projT / v_projT over token chunks.
            # psum zero regions are 2KB per partition row: the two heads
            # sharing (tile, partition half) share one accumulation
            # group -> start on fb==0 head, stop on fb==256 head.
            for h in range(HG):
                i, pb, fb = hloc(h)
                nc.tensor.matmul(
                    kprojT_ps[i][pb:pb + 64, fb:fb + 256],
                    kvt[:, h * 64:(h + 1) * 64],
                    eg[:, h * K:(h + 1) * K],
                    start=(tc_i == 0 and fb == 0),
                    stop=(tc_i == NTC - 1 and fb == 256),
                    skip_group_check=True)
            for h in range(HG):
                i, pb, fb = hloc(h)
                nc.tensor.matmul(
                    vprojT_ps[i][pb:pb + 64, fb:fb + 256],
                    kvt[:, 512 + h * 64: 512 + (h + 1) * 64],
                    fg[:, h * K:(h + 1) * K],
                    start=(tc_i == 0 and fb == 0),
                    stop=(tc_i == NTC - 1 and fb == 256),
                    skip_group_check=True)

        kprojT_sb = [projp.tile([128, 512], f16, name=f"kprojT{i}")
                     for i in range(2)]
        vprojT_sb = [projp.tile([128, 512], f16, name=f"vprojT{i}")
                     for i in range(2)]
        for i in range(2):
            nc.scalar.copy(kprojT_sb[i][:], kprojT_ps[i][:])
            nc.scalar.copy(vprojT_sb[i][:], vprojT_ps[i][:])
        st["kprojT_sb"] = kprojT_sb
        st["vprojT_sb"] = vprojT_sb

    def phase_vp(tc):
        ident, projp = st["ident"], st["projp"]
        vprojT_sb, psvp = st["vprojT_sb"], st["psvp"]
        # vproj_sb[kc]: [128 K-chunk, 8 heads * 64 hd] fp16
        vproj_sb = [projp.tile([128, 512], f16, name=f"vproj{i}")
                    for i in range(2)]
        pvp = [psvp.tile([128, 512], f16, name=f"pvp{kc}") for kc in range(2)]
        for i in range(2):
            for c in range(4):
                h0 = 4 * i + (c // 2) * 2
                nc.tensor.transpose(
                    pvp[c % 2][:, h0 * 64: h0 * 64 + 128],
                    vprojT_sb[i][:, c * 128:(c + 1) * 128],
                    ident[:])
        for kc in range(2):
            nc.scalar.copy(vproj_sb[kc][:], pvp[kc][:])
        st["vproj_sb"] = vproj_sb

    def phase_attn(tc):
        ident = st["ident"]
        kprojT_sb, vproj_sb = st["kprojT_sb"], st["vproj_sb"]
        q_s, wout_s = st["q_s"], st["wout_s"]
        pep, atp, hgp, ysbp, vecp = (st["pep"], st["atp"], st["hgp"],
                                     st["ysbp"], st["vecp"])
        pss, psat, psho, psy = st["pss"], st["psat"], st["psho"], st["psy"]

        def issue_scores_softmax(g, hp):
            """Scores + softmax for one head pair; returns the pe tiles."""
            pes, stats = [], []
            den8 = vecp.tile([128, 8], f32, name="den8")
            r8 = vecp.tile([128, 8], f32, name="r8")
            for half in range(2):
                # one psum bank per head (a bank must only ever be
                # written from one PE row position -- mixing faults HW)
                pe2 = [pep.tile([128, 512], f16, name=f"pe{j}",
                                tag="pe") for j in range(2)]
                for hs in range(2):
                    h = hp * 2 + hs
                    i, pb, fb = hloc(h)
                    qc = h // 2
                    ps = pss.tile([128, 512], f32, name="ps")
                    for t2 in range(2):
                        t = half * 2 + t2
                        nc.tensor.matmul(
                            ps[:, t2 * 256:(t2 + 1) * 256],
                            q_s[pb:pb + 64,
                                qc * N + g * TG + t * 128:
                                qc * N + g * TG + (t + 1) * 128],
                            kprojT_sb[i][pb:pb + 64, fb:fb + 256],
                            start=True, stop=True)
                    nm = vecp.tile([128, 2], f32, name="nm")
                    nc.vector.tensor_reduce(
                        nm[:], ps[:].rearrange("p (c k) -> p c k", k=256),
                        axis=AX, op=MAX, negate=True)
                    for t2 in range(2):
                        di = half * 4 + hs * 2 + t2
                        nc.scalar.activation(
                            pe2[t2][:, hs * 256:(hs + 1) * 256],
                            ps[:, t2 * 256:(t2 + 1) * 256],
                            EXP, bias=nm[:, t2:t2 + 1],
                            accum_out=den8[:, di:di + 1])
                    stats.append((pe2, hs))
                pes.extend(pe2)
            nc.vector.reciprocal(r8[:], den8[:])
            for half in range(2):
                for hs in range(2):
                    for t2 in range(2):
                        di = half * 4 + hs * 2 + t2
                        pe2 = pes[half * 2 + t2]
                        nc.vector.tensor_scalar_mul(
                            pe2[:, hs * 256:(hs + 1) * 256],
                            pe2[:, hs * 256:(hs + 1) * 256],
                            r8[:, di:di + 1])
            return pes

        def issue_transpose_av(g, hp, pes, hgt):
            """Attn transposes + AV matmuls for a pair issued earlier.
            All transposes first so each at copy overlaps the other
            head's transposes, then both AVs."""
            pho = psho.tile([128, 512], f32, name="pho")
            ats = []
            for hs in range(2):
                pat = psat.tile([128, 1024], f16, name="pat")
                for t in range(4):
                    for kc in range(2):
                        nc.tensor.matmul(
                            pat[:, kc * 512 + t * 128:
                                kc * 512 + (t + 1) * 128],
                            pes[t][:, hs * 256 + kc * 128:
                                   hs * 256 + (kc + 1) * 128],
                            ident[:],
                            is_transpose=True, start=True, stop=True)
                at = atp.tile([128, 1024], f16, name="at")
                nc.vector.tensor_copy(at[:], pat[:])
                ats.append(at)
            for hs in range(2):
                h = hp * 2 + hs
                for kc in range(2):
                    nc.tensor.matmul(
                        pho[hs * 64:(hs + 1) * 64, :],
                        vproj_sb[kc][:, h * 64:(h + 1) * 64],
                        ats[hs][:, kc * 512:(kc + 1) * 512],
                        start=(kc == 0), stop=(kc == 1),
                        skip_group_check=True)
            nc.vector.tensor_copy(hgt[hp][:], pho[:])

        issue_q_quarter = st["issue_q_quarter"]

        def issue_y(g, hgt):
            """Fused output projection for a finished token group."""
            for t in range(4):
                ysb = ysbp.tile([128, 1024], f16, name="ysb")
                # ec-blocked: the ec0 bank is copied out while ec1's
                # matmuls run, so the next t's matmuls don't stall
                for ec in range(2):
                    py = psy.tile([128, 512], f32, name="py")
                    for hp in range(4):
                        nc.tensor.matmul(
                            py[:],
                            hgt[hp][:, t * 128:(t + 1) * 128],
                            wout_s[:, hp * DIM + ec * 512:
                                   hp * DIM + (ec + 1) * 512],
                            start=(hp == 0), stop=(hp == 3))
                    nc.scalar.copy(ysb[:, ec * 512:(ec + 1) * 512], py[:])
                nc.sync.dma_start(
                    y[(g * 4 + t) * 128:(g * 4 + t + 1) * 128, :], ysb[:])

        # software-pipelined: scores/softmax of pair i issue before the
        # transposes/AV of pair i-1, so the PE never waits on the softmax
        # chain; y projection of a group issues right after its last AV.
        from collections import deque
        hgts = {}
        queue = deque()
        y_ready = None
        DEPTH = 5
        for qc in range(4):
            issue_q_quarter(0, qc)
        for qc in range(4):
            issue_q_quarter(1, qc)
        pairs = [(g, hp) for g in range(NGRP) for hp in range(4)]
        for g, hp in pairs + [(None, None)] * (DEPTH + 1):
            if g is not None:
                if hp == 0:
                    hgts[g] = [hgp.tile([128, TG], f16, name=f"hgt{i}")
                               for i in range(4)]
                queue.append((g, hp, issue_scores_softmax(g, hp)))
                if g + 2 < NGRP:
                    issue_q_quarter(g + 2, hp)
            if len(queue) > DEPTH or (g is None and queue):
                pg, php, ppes = queue.popleft()
                issue_transpose_av(pg, php, ppes, hgts[pg])
                if php == 3:
                    y_ready = pg
                    continue
            if y_ready is not None:
                issue_y(y_ready, hgts.pop(y_ready))
                y_ready = None


    with tile.TileContext(nc) as tc:
        with (
            tc.tile_pool(name="const", bufs=1) as constp,
            tc.tile_pool(name="persist", bufs=1) as persistp,
            tc.tile_pool(name="proj_sb", bufs=1) as projp,
        ):
            ident = constp.tile([128, 128], f16)
            make_identity(nc, ident[:])
            st["ident"] = ident
            st["projp"] = projp

            x_s = persistp.tile([128, 8 * N], f16, name="x_s")
            wq_s = persistp.tile([128, 8 * HGD], f16, name="wq_s")
            wkv_s = persistp.tile([128, 8 * 2 * HGD], f16, name="wkv_s")
            wout_s = persistp.tile([128, 4 * DIM], f16, name="wout_s")
            q_s = persistp.tile([128, 4 * N], f16, name="q_s")
            st.update(x_s=x_s, wq_s=wq_s, wkv_s=wkv_s, wout_s=wout_s, q_s=q_s)

            def load_x_group(g):
                for dc in range(8):
                    nc.sync.dma_start(
                        x_s[:, dc * N + g * TG: dc * N + (g + 1) * TG],
                        xT[dc * 128:(dc + 1) * 128, g * TG:(g + 1) * TG])

            # wkv + first x groups up front; the rest stream from inside
            # phase_kv so the E/F stream isn't starved at startup
            # startup order tuned so the first chunk's matmuls (k: wkv
            # k-half + x group 0, then proj: eg/fg chunk 0) wait on the
            # fewest possible bytes
            for dc in range(8):
                nc.sync.dma_start(wkv_s[:, dc * 2 * HGD: dc * 2 * HGD + HGD],
                                  wkvT[dc * 128:(dc + 1) * 128, 0:HGD])
            for dc in range(8):
                nc.sync.dma_start(x_s[:, dc * N: dc * N + 128],
                                  xT[dc * 128:(dc + 1) * 128, 0:128])
            for dc in range(8):
                nc.sync.dma_start(x_s[:, dc * N + 128: dc * N + TG],
                                  xT[dc * 128:(dc + 1) * 128, 128:TG])
            st["ef0"] = ef0 = projp.tile([128, 2 * HG * K], f16, name="ef0")
            nc.sync.dma_start(ef0[:, 0:HG * K], eT[0:128, :])
            for dc in range(8):
                nc.sync.dma_start(
                    wkv_s[:, dc * 2 * HGD + HGD:(dc + 1) * 2 * HGD],
                    wkvT[dc * 128:(dc + 1) * 128, HGD:2 * HGD])
            nc.sync.dma_start(ef0[:, HG * K:2 * HG * K], fT[0:128, :])
            load_x_group(1)
            st["load_x_group"] = load_x_group

            psq_ctx = tc.tile_pool(name="ps_q", bufs=1, space="PSUM")
            st["psq"] = psq_ctx.__enter__()
            with (
                tc.tile_pool(name="ef", bufs=3) as efp,
                tc.tile_pool(name="kv", bufs=3) as kvp,
                tc.tile_pool(name="ps_kv", bufs=1, space="PSUM") as pskv,
                tc.tile_pool(name="ps_proj", bufs=1, space="PSUM") as pspr,
            ):
                st.update(efp=efp, kvp=kvp, pskv=pskv, pspr=pspr)
                phase_kv(tc)

            if level >= 2:
                with tc.tile_pool(name="ps_vp", bufs=1, space="PSUM") as psvp:
                    st["psvp"] = psvp
                    phase_vp(tc)

            if level >= 4:
              with (
                tc.tile_pool(name="pe", bufs=28) as pep,
                tc.tile_pool(name="at", bufs=6) as atp,
                tc.tile_pool(name="hgt", bufs=3) as hgp,
                tc.tile_pool(name="ysb", bufs=3) as ysbp,
                tc.tile_pool(name="vec", bufs=10) as vecp,
                tc.tile_pool(name="ps_s", bufs=2, space="PSUM") as pss,
                tc.tile_pool(name="ps_at", bufs=2, space="PSUM") as psat,
                tc.tile_pool(name="ps_ho", bufs=1, space="PSUM") as psho,
                tc.tile_pool(name="ps_y", bufs=2, space="PSUM") as psy,
            ):
                st.update(pep=pep, atp=atp, hgp=hgp, ysbp=ysbp, vecp=vecp,
                          pss=pss, psat=psat, psho=psho, psy=psy)
                phase_attn(tc)

            psq_ctx.__exit__(None, None, None)

    if level < 4:
        with tile.TileContext(nc) as tc2:
            with tc2.tile_pool(name="dummy", bufs=1) as dp:
                zt = dp.tile([128, DIM], f16)
                nc.gpsimd.memset(zt[:], 0.0)
                for gb in range(NTC):
                    nc.sync.dma_start(y[gb * 128:(gb + 1) * 128, :], zt[:])

    nc.compile()
    return nc


def _prep_inputs(x, Wqkv, E, F, Wout):
    """Build the 8 per-core input dicts (host-side slicing/transposes)."""
    f16 = np.float16
    ins = []
    per_hg = {}
    for hg in range(2):
        r = hg * HGD
        wqT = np.ascontiguousarray(Wqkv[r:r + HGD, :].T, dtype=f16)
        wk = Wqkv[DIM + r: DIM + r + HGD, :]
        wv = Wqkv[2 * DIM + r: 2 * DIM + r + HGD, :]
        wkvT = np.concatenate([wk.T, wv.T], axis=1).astype(f16)
        # E pre-scaled by SCALE so scores come out of the PE pre-scaled
        eT = np.ascontiguousarray(
            E[hg * HG:(hg + 1) * HG].transpose(2, 0, 1).reshape(N, HG * K)
            * SCALE, dtype=f16)
        fT = np.ascontiguousarray(
            F[hg * HG:(hg + 1) * HG].transpose(2, 0, 1).reshape(N, HG * K),
            dtype=f16)
        woutT = np.ascontiguousarray(Wout[:, r:r + HGD].T, dtype=f16)
        per_hg[hg] = (wqT, wkvT, eT, fT, woutT)
    xTs = [np.ascontiguousarray(x[b].T, dtype=f16) for b in range(B)]
    for c in range(NCORES):
        b, hg = c // 2, c % 2
        wqT, wkvT, eT, fT, woutT = per_hg[hg]
        ins.append({"xT": xTs[b], "wqT": wqT, "wkvT": wkvT,
                    "eT": eT, "fT": fT, "woutT": woutT})
    return ins


def kernel(x, Wqkv, E, F, Wout, bout):
    from concourse.bass_utils import run_bass_kernel_spmd

    x = np.asarray(x, dtype=np.float32)
    Wqkv = np.asarray(Wqkv, dtype=np.float32)
    E = np.asarray(E, dtype=np.float32)
    F = np.asarray(F, dtype=np.float32)
    Wout = np.asarray(Wout, dtype=np.float32)
    bout = np.asarray(bout, dtype=np.float32)

    if "nc" not in _cache:
        _cache["nc"] = _build()
    nc = _cache["nc"]

    in_maps = _prep_inputs(x, Wqkv, E, F, Wout)
    res = run_bass_kernel_spmd(nc, in_maps, core_ids=list(range(NCORES)))
    out = np.empty((B, N, DIM), dtype=np.float32)
    for b in range(B):
        out[b] = (res.results[2 * b]["y"].astype(np.float32)
                  + res.results[2 * b + 1]["y"].astype(np.float32) + bout)
    return out



# revision 10
# speedup vs baseline: 3.0408x; 1.0072x over previous
"""Linformer self-attention Trainium2 kernel (fp16 PE pipeline).

Problem (hardcoded): B=4, N=4096, DIM=1024, H=16, K=256, HD=64, fp32 I/O.
  qkv = x @ Wqkv.T; q,k,v split into 16 heads of 64
  k_proj = E @ k, v_proj = F @ v  (per head, contract over tokens)
  out = softmax(q @ k_proj.T / 8) @ v_proj
  y = out @ Wout.T + bout

Sharding: 8 cores = (batch b = c//2) x (head-group hg = c%2, 8 heads each).
Each core computes a (4096, 1024) fp16 partial of y for its batch; host
sums hg=0 + hg=1 partials in fp32 and adds bout. No collectives.

All matmuls run in fp16 (1 cycle/row on the PE vs 4 for fp32; end-to-end
rel err ~5e-3). Per core:
  x resident in SBUF fp16 [128, 8*4096] (xdim-chunk major); startup DMA
    order tuned so the first k/v chunk waits on the fewest bytes.
  Phase A: k,v per 128-token chunk (tokens on partitions), immediately
    contracted with streamed E/F chunks into k_projT/v_projT psum
    accumulators (8 heads packed into 2+2 banks); x groups and the
    q/out weights stream in from inside the chunk loop; the last 8
    chunks also interleave the first two groups of q as PE filler.
  Phase A15: v_projT transposed to vproj [K on partitions] via PE.
  Phase C (software-pipelined over 32 head-pair slots): scores for a
    pair land in one psum bank per head (a psum bank must only ever be
    written from one PE row position -- mixing row positions faults the
    hardware); batched DVE max -> ACT exp (fused scale/bias, denominator
    via accum_out) -> DVE reciprocal+normalize; the fp16 attn transposes
    + AV matmuls for a pair issue DEPTH=3 slots later so the PE always
    has work while the softmax chain runs; q for group g+2 interleaves
    as filler; the output projection for a finished group issues one
    slot after its last AV and streams y out as fp16.
"""

import numpy as np

B, N, DIM, H, K = 4, 4096, 1024, 16, 256
HD = DIM // H
SCALE = 1.0 / 8.0
HG = H // 2          # 8 heads per core
HGD = HG * HD        # 512 head dims per core
NCORES = 8
TG = 512             # token group
NTC = N // 128       # 32 token chunks
NGRP = N // TG       # 8 token groups

_cache = {}


def _build(level=4):
    import concourse.mybir as mybir
    import concourse.tile as tile
    from concourse import bacc
    from concourse.masks import make_identity

    f32 = mybir.dt.float32
    f16 = mybir.dt.float16
    AX = mybir.AxisListType.X
    MAX = mybir.AluOpType.max
    EXP = mybir.ActivationFunctionType.Exp

    nc = bacc.Bacc("TRN2", target_bir_lowering=False, debug=False,
                   enable_asserts=False)

    xT = nc.dram_tensor("xT", (DIM, N), f16, kind="ExternalInput").ap()
    wqT = nc.dram_tensor("wqT", (DIM, HGD), f16, kind="ExternalInput").ap()
    wkvT = nc.dram_tensor("wkvT", (DIM, 2 * HGD), f16, kind="ExternalInput").ap()
    eT = nc.dram_tensor("eT", (N, HG * K), f16, kind="ExternalInput").ap()
    fT = nc.dram_tensor("fT", (N, HG * K), f16, kind="ExternalInput").ap()
    woutT = nc.dram_tensor("woutT", (HGD, DIM), f16, kind="ExternalInput").ap()
    y = nc.dram_tensor("y", (N, DIM), f16, kind="ExternalOutput").ap()

    def hloc(h):
        # head h (0..7) -> (tile idx, partition base, free base) in the
        # packed [128, 512] projT psum/sbuf tiles
        sub = h % 4
        return h // 4, (sub % 2) * 64, (sub // 2) * 256

    st = {}

    def issue_q_quarter(g, qc):
        """One qdim-chunk of q for a future group -- PE filler work
        interleaved with other phases."""
        x_s, wq_s, q_s, psq = st["x_s"], st["wq_s"], st["q_s"], st["psq"]
        pq = psq.tile([128, 512], f32, name="pq")
        for dc in range(8):
            nc.tensor.matmul(
                pq[:],
                wq_s[:, dc * HGD + qc * 128: dc * HGD + (qc + 1) * 128],
                x_s[:, dc * N + g * TG: dc * N + (g + 1) * TG],
                start=(dc == 0), stop=(dc == 7))
        nc.vector.tensor_copy(
            q_s[:, qc * N + g * TG: qc * N + (g + 1) * TG], pq[:])
    st["issue_q_quarter"] = issue_q_quarter

    def phase_kv(tc):
        efp, kvp = st["efp"], st["kvp"]
        pskv, pspr = st["pskv"], st["pspr"]
        x_s, wkv_s = st["x_s"], st["wkv_s"]
        projp = st["projp"]

        kprojT_ps = [pspr.tile([128, 512], f32, name=f"kprojTps{i}")
                     for i in range(2)]
        vprojT_ps = [pspr.tile([128, 512], f32, name=f"vprojTps{i}")
                     for i in range(2)]

        wq_s, wout_s = st["wq_s"], st["wout_s"]
        for tc_i in range(NTC):
            if tc_i == 12:
                for dc in range(8):
                    nc.sync.dma_start(wq_s[:, dc * HGD:(dc + 1) * HGD],
                                      wqT[dc * 128:(dc + 1) * 128, :])
            if tc_i == 16:
                for dc in range(4):
                    nc.sync.dma_start(wout_s[:, dc * DIM:(dc + 1) * DIM],
                                      woutT[dc * 128:(dc + 1) * 128, :])

            if tc_i == 0:
                ef0 = st["ef0"]
                eg = ef0[:, 0:HG * K]
                fg = ef0[:, HG * K:2 * HG * K]
            else:
                eg_t = efp.tile([128, HG * K], f16, name="eg")
                fg_t = efp.tile([128, HG * K], f16, name="fg")
                nc.sync.dma_start(eg_t[:], eT[tc_i * 128:(tc_i + 1) * 128, :])
                nc.sync.dma_start(fg_t[:], fT[tc_i * 128:(tc_i + 1) * 128, :])
                eg, fg = eg_t[:], fg_t[:]
            # x prefetch AFTER the E/F issue so it never queues ahead of
            # the stream the proj matmuls wait on
            g2 = tc_i // 4 + 2
            if g2 < NGRP:
                for dc in (2 * (tc_i % 4), 2 * (tc_i % 4) + 1):
                    nc.sync.dma_start(
                        x_s[:, dc * N + g2 * TG: dc * N + (g2 + 1) * TG],
                        xT[dc * 128:(dc + 1) * 128, g2 * TG:(g2 + 1) * TG])
            pk = pskv.tile([128, 512], f32, name="pk")
            pv = pskv.tile([128, 512], f32, name="pv")
            for dc in range(8):
                xc = x_s[:, dc * N + tc_i * 128: dc * N + (tc_i + 1) * 128]
                nc.tensor.matmul(pk[:], xc,
                                 wkv_s[:, dc * 1024: dc * 1024 + 512],
                                 start=(dc == 0), stop=(dc == 7))
            kvt = kvp.tile([128, 1024], f16)
            nc.scalar.copy(kvt[:, 0:512], pk[:])
            for dc in range(8):
                xc = x_s[:, dc * N + tc_i * 128: dc * N + (tc_i + 1) * 128]
                nc.tensor.matmul(pv[:], xc,
                                 wkv_s[:, dc * 1024 + 512: dc * 1024 + 1024],
                                 start=(dc == 0), stop=(dc == 7))
            nc.scalar.copy(kvt[:, 512:1024], pv[:])
            # accumulate k_projT / v_projT over token chunks.
            # psum zero regions are 2KB per partition row: the two heads
            # sharing (tile, partition half) share one accumulation
            # group -> start on fb==0 head, stop on fb==256 head.
            for h in range(HG):
                i, pb, fb = hloc(h)
                nc.tensor.matmul(
                    kprojT_ps[i][pb:pb + 64, fb:fb + 256],
                    kvt[:, h * 64:(h + 1) * 64],
                    eg[:, h * K:(h + 1) * K],
                    start=(tc_i == 0 and fb == 0),
                    stop=(tc_i == NTC - 1 and fb == 256),
                    skip_group_check=True)
            for h in range(HG):
                i, pb, fb = hloc(h)
                nc.tensor.matmul(
                    vprojT_ps[i][pb:pb + 64, fb:fb + 256],
                    kvt[:, 512 + h * 64: 512 + (h + 1) * 64],
                    fg[:, h * K:(h + 1) * K],
                    start=(tc_i == 0 and fb == 0),
                    stop=(tc_i == NTC - 1 and fb == 256),
                    skip_group_check=True)

        kprojT_sb = [projp.tile([128, 512], f16, name=f"kprojT{i}")
                     for i in range(2)]
        vprojT_sb = [projp.tile([128, 512], f16, name=f"vprojT{i}")
                     for i in range(2)]
        for i in range(2):
            nc.scalar.copy(kprojT_sb[i][:], kprojT_ps[i][:])
            nc.scalar.copy(vprojT_sb[i][:], vprojT_ps[i][:])
        st["kprojT_sb"] = kprojT_sb
        st["vprojT_sb"] = vprojT_sb

    def phase_vp(tc):
        ident, projp = st["ident"], st["projp"]
        vprojT_sb, psvp = st["vprojT_sb"], st["psvp"]
        # vproj_sb[kc]: [128 K-chunk, 8 heads * 64 hd] fp16
        vproj_sb = [projp.tile([128, 512], f16, name=f"vproj{i}")
                    for i in range(2)]
        pvp = [psvp.tile([128, 512], f16, name=f"pvp{kc}") for kc in range(2)]
        for i in range(2):
            for c in range(4):
                h0 = 4 * i + (c // 2) * 2
                nc.tensor.transpose(
                    pvp[c % 2][:, h0 * 64: h0 * 64 + 128],
                    vprojT_sb[i][:, c * 128:(c + 1) * 128],
                    ident[:])
        for kc in range(2):
            nc.scalar.copy(vproj_sb[kc][:], pvp[kc][:])
        st["vproj_sb"] = vproj_sb

    def phase_attn(tc):
        ident = st["ident"]
        kprojT_sb, vproj_sb = st["kprojT_sb"], st["vproj_sb"]
        q_s, wout_s = st["q_s"], st["wout_s"]
        pep, atp, hgp, ysbp, vecp = (st["pep"], st["atp"], st["hgp"],
                                     st["ysbp"], st["vecp"])
        pss, psat, psho, psy = st["pss"], st["psat"], st["psho"], st["psy"]

        def issue_scores_softmax(g, hp):
            """Scores + softmax for one head pair; returns the pe tiles."""
            pes, stats = [], []
            den8 = vecp.tile([128, 8], f32, name="den8")
            r8 = vecp.tile([128, 8], f32, name="r8")
            for half in range(2):
                # one psum bank per head (a bank must only ever be
                # written from one PE row position -- mixing faults HW)
                pe2 = [pep.tile([128, 512], f16, name=f"pe{j}",
                                tag="pe") for j in range(2)]
                for hs in range(2):
                    h = hp * 2 + hs
                    i, pb, fb = hloc(h)
                    qc = h // 2
                    ps = pss.tile([128, 512], f32, name="ps")
                    for t2 in range(2):
                        t = half * 2 + t2
                        nc.tensor.matmul(
                            ps[:, t2 * 256:(t2 + 1) * 256],
                            q_s[pb:pb + 64,
                                qc * N + g * TG + t * 128:
                                qc * N + g * TG + (t + 1) * 128],
                            kprojT_sb[i][pb:pb + 64, fb:fb + 256],
                            start=True, stop=True)
                    nm = vecp.tile([128, 2], f32, name="nm")
                    nc.vector.tensor_reduce(
                        nm[:], ps[:].rearrange("p (c k) -> p c k", k=256),
                        axis=AX, op=MAX, negate=True)
                    for t2 in range(2):
                        di = half * 4 + hs * 2 + t2
                        nc.scalar.activation(
                            pe2[t2][:, hs * 256:(hs + 1) * 256],
                            ps[:, t2 * 256:(t2 + 1) * 256],
                            EXP, bias=nm[:, t2:t2 + 1],
                            accum_out=den8[:, di:di + 1])
                    stats.append((pe2, hs))
                pes.extend(pe2)
            nc.vector.reciprocal(r8[:], den8[:])
            for half in range(2):
                for hs in range(2):
                    for t2 in range(2):
                        di = half * 4 + hs * 2 + t2
                        pe2 = pes[half * 2 + t2]
                        nc.vector.tensor_scalar_mul(
                            pe2[:, hs * 256:(hs + 1) * 256],
                            pe2[:, hs * 256:(hs + 1) * 256],
                            r8[:, di:di + 1])
            return pes

        def issue_transpose_av(g, hp, pes, hgt):
            """Attn transposes + AV matmuls for a pair issued earlier.
            All transposes first so each at copy overlaps the other
            head's transposes, then both AVs."""
            pho = psho.tile([128, 512], f32, name="pho")
            ats = []
            for hs in range(2):
                pat = psat.tile([128, 1024], f16, name="pat")
                for t in range(4):
                    for kc in range(2):
                        nc.tensor.matmul(
                            pat[:, kc * 512 + t * 128:
                                kc * 512 + (t + 1) * 128],
                            pes[t][:, hs * 256 + kc * 128:
                                   hs * 256 + (kc + 1) * 128],
                            ident[:],
                            is_transpose=True, start=True, stop=True)
                at = atp.tile([128, 1024], f16, name="at")
                nc.vector.tensor_copy(at[:], pat[:])
                ats.append(at)
            for hs in range(2):
                h = hp * 2 + hs
                for kc in range(2):
                    nc.tensor.matmul(
                        pho[hs * 64:(hs + 1) * 64, :],
                        vproj_sb[kc][:, h * 64:(h + 1) * 64],
                        ats[hs][:, kc * 512:(kc + 1) * 512],
                        start=(kc == 0), stop=(kc == 1),
                        skip_group_check=True)
            nc.vector.tensor_copy(hgt[hp][:], pho[:])

        issue_q_quarter = st["issue_q_quarter"]

        def issue_y(g, hgt):
            """Fused output projection for a finished token group."""
            for t in range(4):
                ysb = ysbp.tile([128, 1024], f16, name="ysb")
                # ec-blocked: the ec0 bank is copied out while ec1's
                # matmuls run, so the next t's matmuls don't stall
                for ec in range(2):
                    py = psy.tile([128, 512], f32, name="py")
                    for hp in range(4):
                        nc.tensor.matmul(
                            py[:],
                            hgt[hp][:, t * 128:(t + 1) * 128],
                            wout_s[:, hp * DIM + ec * 512:
                                   hp * DIM + (ec + 1) * 512],
                            start=(hp == 0), stop=(hp == 3))
                    if ec == 0:
                        nc.scalar.copy(ysb[:, 0:512], py[:])
                    else:
                        nc.vector.tensor_copy(ysb[:, 512:1024], py[:])
                nc.sync.dma_start(
                    y[(g * 4 + t) * 128:(g * 4 + t + 1) * 128, :], ysb[:])

        # software-pipelined: scores/softmax of pair i issue before the
        # transposes/AV of pair i-1, so the PE never waits on the softmax
        # chain; y projection of a group issues right after its last AV.
        from collections import deque
        hgts = {}
        queue = deque()
        y_ready = None
        DEPTH = 5
        for qc in range(4):
            issue_q_quarter(0, qc)
        for qc in range(4):
            issue_q_quarter(1, qc)
        pairs = [(g, hp) for g in range(NGRP) for hp in range(4)]
        for g, hp in pairs + [(None, None)] * (DEPTH + 1):
            if g is not None:
                if hp == 0:
                    hgts[g] = [hgp.tile([128, TG], f16, name=f"hgt{i}")
                               for i in range(4)]
                queue.append((g, hp, issue_scores_softmax(g, hp)))
                if g + 2 < NGRP:
                    issue_q_quarter(g + 2, hp)
            if len(queue) > DEPTH or (g is None and queue):
                pg, php, ppes = queue.popleft()
                issue_transpose_av(pg, php, ppes, hgts[pg])
                if php == 3:
                    y_ready = pg
                    continue
            if y_ready is not None:
                issue_y(y_ready, hgts.pop(y_ready))
                y_ready = None


    with tile.TileContext(nc) as tc:
        with (
            tc.tile_pool(name="const", bufs=1) as constp,
            tc.tile_pool(name="persist", bufs=1) as persistp,
            tc.tile_pool(name="proj_sb", bufs=1) as projp,
        ):
            ident = constp.tile([128, 128], f16)
            make_identity(nc, ident[:])
            st["ident"] = ident
            st["projp"] = projp

            x_s = persistp.tile([128, 8 * N], f16, name="x_s")
            wq_s = persistp.tile([128, 8 * HGD], f16, name="wq_s")
            wkv_s = persistp.tile([128, 8 * 2 * HGD], f16, name="wkv_s")
            wout_s = persistp.tile([128, 4 * DIM], f16, name="wout_s")
            q_s = persistp.tile([128, 4 * N], f16, name="q_s")
            st.update(x_s=x_s, wq_s=wq_s, wkv_s=wkv_s, wout_s=wout_s, q_s=q_s)

            def load_x_group(g):
                for dc in range(8):
                    nc.sync.dma_start(
                        x_s[:, dc * N + g * TG: dc * N + (g + 1) * TG],
                        xT[dc * 128:(dc + 1) * 128, g * TG:(g + 1) * TG])

            # wkv + first x groups up front; the rest stream from inside
            # phase_kv so the E/F stream isn't starved at startup
            # startup order tuned so the first chunk's matmuls (k: wkv
            # k-half + x group 0, then proj: eg/fg chunk 0) wait on the
            # fewest possible bytes
            for dc in range(4):
                nc.sync.dma_start(wkv_s[:, dc * 2 * HGD: dc * 2 * HGD + HGD],
                                  wkvT[dc * 128:(dc + 1) * 128, 0:HGD])
            for dc in range(4, 8):
                nc.scalar.dma_start(wkv_s[:, dc * 2 * HGD: dc * 2 * HGD + HGD],
                                    wkvT[dc * 128:(dc + 1) * 128, 0:HGD])
            for dc in range(8):
                nc.scalar.dma_start(x_s[:, dc * N: dc * N + 128],
                                    xT[dc * 128:(dc + 1) * 128, 0:128])
            for dc in range(8):
                nc.sync.dma_start(x_s[:, dc * N + 128: dc * N + TG],
                                  xT[dc * 128:(dc + 1) * 128, 128:TG])
            st["ef0"] = ef0 = projp.tile([128, 2 * HG * K], f16, name="ef0")
            nc.sync.dma_start(ef0[:, 0:HG * K], eT[0:128, :])
            for dc in range(8):
                nc.sync.dma_start(
                    wkv_s[:, dc * 2 * HGD + HGD:(dc + 1) * 2 * HGD],
                    wkvT[dc * 128:(dc + 1) * 128, HGD:2 * HGD])
            nc.sync.dma_start(ef0[:, HG * K:2 * HG * K], fT[0:128, :])
            load_x_group(1)
            st["load_x_group"] = load_x_group

            psq_ctx = tc.tile_pool(name="ps_q", bufs=1, space="PSUM")
            st["psq"] = psq_ctx.__enter__()
            with (
                tc.tile_pool(name="ef", bufs=3) as efp,
                tc.tile_pool(name="kv", bufs=3) as kvp,
                tc.tile_pool(name="ps_kv", bufs=1, space="PSUM") as pskv,
                tc.tile_pool(name="ps_proj", bufs=1, space="PSUM") as pspr,
            ):
                st.update(efp=efp, kvp=kvp, pskv=pskv, pspr=pspr)
                phase_kv(tc)

            if level >= 2:
                with tc.tile_pool(name="ps_vp", bufs=1, space="PSUM") as psvp:
                    st["psvp"] = psvp
                    phase_vp(tc)

            if level >= 4:
              with (
                tc.tile_pool(name="pe", bufs=28) as pep,
                tc.tile_pool(name="at", bufs=6) as atp,
                tc.tile_pool(name="hgt", bufs=3) as hgp,
                tc.tile_pool(name="ysb", bufs=3) as ysbp,
                tc.tile_pool(name="vec", bufs=10) as vecp,
                tc.tile_pool(name="ps_s", bufs=2, space="PSUM") as pss,
                tc.tile_pool(name="ps_at", bufs=2, space="PSUM") as psat,
                tc.tile_pool(name="ps_ho", bufs=1, space="PSUM") as psho,
                tc.tile_pool(name="ps_y", bufs=2, space="PSUM") as psy,
            ):
                st.update(pep=pep, atp=atp, hgp=hgp, ysbp=ysbp, vecp=vecp,
                          pss=pss, psat=psat, psho=psho, psy=psy)
                phase_attn(tc)

            psq_ctx.__exit__(None, None, None)

    if level < 4:
        with tile.TileContext(nc) as tc2:
            with tc2.tile_pool(name="dummy", bufs=1) as dp:
                zt = dp.tile([128, DIM], f16)
                nc.gpsimd.memset(zt[:], 0.0)
                for gb in range(NTC):
                    nc.sync.dma_start(y[gb * 128:(gb + 1) * 128, :], zt[:])

    nc.compile()
    return nc


def _prep_inputs(x, Wqkv, E, F, Wout):
    """Build the 8 per-core input dicts (host-side slicing/transposes)."""
    f16 = np.float16
    ins = []
    per_hg = {}
    for hg in range(2):
        r = hg * HGD
        wqT = np.ascontiguousarray(Wqkv[r:r + HGD, :].T, dtype=f16)
        wk = Wqkv[DIM + r: DIM + r + HGD, :]
        wv = Wqkv[2 * DIM + r: 2 * DIM + r + HGD, :]
        wkvT = np.concatenate([wk.T, wv.T], axis=1).astype(f16)
        # E pre-scaled by SCALE so scores come out of the PE pre-scaled
        eT = np.ascontiguousarray(
            E[hg * HG:(hg + 1) * HG].transpose(2, 0, 1).reshape(N, HG * K)
            * SCALE, dtype=f16)
        fT = np.ascontiguousarray(
            F[hg * HG:(hg + 1) * HG].transpose(2, 0, 1).reshape(N, HG * K),
            dtype=f16)
        woutT = np.ascontiguousarray(Wout[:, r:r + HGD].T, dtype=f16)
        per_hg[hg] = (wqT, wkvT, eT, fT, woutT)
    xTs = [np.ascontiguousarray(x[b].T, dtype=f16) for b in range(B)]
    for c in range(NCORES):
        b, hg = c // 2, c % 2
        wqT, wkvT, eT, fT, woutT = per_hg[hg]
        ins.append({"xT": xTs[b], "wqT": wqT, "wkvT": wkvT,
                    "eT": eT, "fT": fT, "woutT": woutT})
    return ins


def kernel(x, Wqkv, E, F, Wout, bout):
    from concourse.bass_utils import run_bass_kernel_spmd

    x = np.asarray(x, dtype=np.float32)
    Wqkv = np.asarray(Wqkv, dtype=np.float32)
    E = np.asarray(E, dtype=np.float32)
    F = np.asarray(F, dtype=np.float32)
    Wout = np.asarray(Wout, dtype=np.float32)
    bout = np.asarray(bout, dtype=np.float32)

    if "nc" not in _cache:
        _cache["nc"] = _build()
    nc = _cache["nc"]

    in_maps = _prep_inputs(x, Wqkv, E, F, Wout)
    res = run_bass_kernel_spmd(nc, in_maps, core_ids=list(range(NCORES)))
    out = np.empty((B, N, DIM), dtype=np.float32)
    for b in range(B):
        out[b] = (res.results[2 * b]["y"].astype(np.float32)
                  + res.results[2 * b + 1]["y"].astype(np.float32) + bout)
    return out

